# revision 52
# baseline (speedup 1.0000x reference)
"""DeepseekMoE Trainium2 Bass kernel (8-core expert-parallel, v5).

kernel(**inputs) takes FULL unsharded inputs (as produced by setup_inputs)
and returns the FULL output [1, 2048, 1024] fp32.

Sharding (8 cores):
  - Expert-parallel: 2 of 16 experts per core.
  - Shared expert: F-dim sliced 1408/8=176 per core (partial sums).
  - Router replicated per core.
  - Host: out = x + sum(per-core partials).

v5 design:
  - fp16 compute, fp8(e4m3) DoubleRow matmuls for dispatch and expert
    gate/up (weights pre-scaled x64 on host; the descale rides through
    silu's scale arg and a 1/64 fold into w_down).
  - Zero gpsimd usage (its tensor ops are ~20x slow AND stall the vector
    engine); iotas/identity are host inputs.
  - Sel (token-major 0/1) built per tile with scalar-engine 2-pass
    relu(1-|iota-gf0|) (even tiles) or vector is_equal (odd tiles),
    + 1 vector STT ((iota==gf1)+sel0).
  - SelCT (slot-major, prob-weighted) built directly via STT with the
    per-partition slot id as scalar, against PE-broadcast gf/w rows;
    scheduled inside the dispatch window, off the critical path.
  - Routing streamed in 2 groups of 8 token tiles (chunked in-place scan).
  - Dispatch as 4 passes (expert x d-half), 4 PSUM banks each, fp8
    DoubleRow over token-tile pairs, streaming behind Sel production.
  - Expert weights host-packed fp8: one DMA descriptor per (expert,
    F-half) for gate+up, one fp16 per expert for down; streamed through
    2-buffer pools.
  - Partial output in fp16.
"""
import numpy as np

# ---- problem constants (hardcoded; kernel.py must be self-contained) ----
N = 2048          # tokens
D = 1024          # model dim
E = 16            # experts
F = 1408          # expert ffn dim
C = 320           # per-expert capacity = ceil(1.25 * N*K / E)
NCORES = 8
EPC = E // NCORES  # experts per core = 2
FSH = F // NCORES  # shared-expert F slice = 176
P = 128
NT = N // P        # 16 token tiles
DC = D // P        # 8 d-chunks
FC = F // P        # 11 f-chunks
NSLOT = EPC * C    # 640 slots per core
TRASH = NSLOT      # sentinel slot id (matches nothing in iota 0..639)
ST = NSLOT // P    # 5 slot tiles
NG = 2             # routing groups
GT = NT // NG      # tiles per group = 8
FH = (768, 640)    # expert F halves (6 + 5 chunks of 128)
HW2 = 2 * 768      # per-d-chunk packed row width (h1 padded to match)
WSC = 64.0         # fp8 weight pre-scale (keeps w in e4m3 normal range)

_BUILD_CACHE = {}


def _build_nc():
    import concourse.bacc as bacc
    import concourse.mybir as mybir
    import concourse.tile as tile

    f32 = mybir.dt.float32
    fp16 = mybir.dt.float16
    fp8 = mybir.dt.float8e4
    u32 = mybir.dt.uint32
    Alu = mybir.AluOpType
    Act = mybir.ActivationFunctionType
    DR = mybir.MatmulPerfMode.DoubleRow

    nc = bacc.Bacc("TRN2", target_bir_lowering=False, debug=False)

    # ---- I/O (all host-packed) ----
    xp = nc.dram_tensor("xp", [P, NT, D], fp8, kind="ExternalInput").ap()
    xtp = nc.dram_tensor("xtp", [P, DC * N], fp16, kind="ExternalInput").ap()
    rwp = nc.dram_tensor("rwp", [P, DC * E], fp16, kind="ExternalInput").ap()
    rbp = nc.dram_tensor("rbp", [1, E], fp16, kind="ExternalInput").ap()
    whp = nc.dram_tensor("whp", [EPC * 2, P, DC, HW2], fp8,
                         kind="ExternalInput").ap()
    wdp = nc.dram_tensor("wdp", [EPC, P, FC, D], fp8,
                         kind="ExternalInput").ap()
    swgp = nc.dram_tensor("swgp", [P, DC * FSH], fp16,
                          kind="ExternalInput").ap()
    swup = nc.dram_tensor("swup", [P, DC * FSH], fp16,
                          kind="ExternalInput").ap()
    swdp = nc.dram_tensor("swdp", [FSH, D], fp16, kind="ExternalInput").ap()
    coff = nc.dram_tensor("coff", [P, 1], f32, kind="ExternalInput").ap()
    pcol = nc.dram_tensor("pcol", [P, 1], f32, kind="ExternalInput").ap()
    iotaE = nc.dram_tensor("iotaE", [P, E], f32, kind="ExternalInput").ap()
    iotaS = nc.dram_tensor("iotaS", [P, NSLOT], fp16,
                           kind="ExternalInput").ap()
    identf = nc.dram_tensor("identf", [P, P], f32, kind="ExternalInput").ap()
    partial = nc.dram_tensor("partial", [N, D], fp16,
                             kind="ExternalOutput").ap()

    FSH_CH = [(0, P), (P, FSH - P)]        # shared-expert f chunks: 128 + 48
    # per-expert slot sub-chunks (offset within expert, rows) and the
    # global y-tile/row they land in
    ECH = {0: [(0, 128, 0, 0), (128, 128, 1, 0), (256, 64, 2, 0)],
           1: [(0, 64, 2, 64), (64, 128, 3, 0), (192, 128, 4, 0)]}

    with tile.TileContext(nc) as tc:
        with tc.tile_pool(name="persist", bufs=1) as pp, \
             tc.tile_pool(name="stream", bufs=4) as sp:

            # ---- tiles for constants (DMAs issued after xT below) ----
            ident = pp.tile([P, P], f32, tag="ident")
            iota_e = pp.tile([P, E], f32, tag="iota_e")
            iota_h = pp.tile([P, NSLOT], fp16, tag="iota_h")
            coff_t = pp.tile([P, 1], f32, tag="coff_t")
            pcol_t = pp.tile([P, 1], f32, tag="pcol_t")
            rw_sb = pp.tile([P, DC * E], fp16, tag="rw_sb")
            rb_sb = pp.tile([1, E], fp16, tag="rb_sb")
            ones_row = pp.tile([1, P], fp16, tag="ones_row")
            nc.vector.memset(ones_row[:], 1.0)
            swg_sb = pp.tile([P, DC * FSH], fp16, tag="swg_sb")
            swu_sb = pp.tile([P, DC * FSH], fp16, tag="swu_sb")
            swd_sb = [pp.tile([fl, D], fp16, tag=f"swd_{f0}",
                              name=f"swd_{f0}") for (f0, fl) in FSH_CH]

            # routing staging [128, NT] (column = token tile), f32
            d01s = pp.tile([P, NT], f32, tag="d01s")
            idx0s = pp.tile([P, NT], f32, tag="idx0s")
            idx1s = pp.tile([P, NT], f32, tag="idx1s")
            pos0s = pp.tile([P, NT], f32, tag="pos0s")
            pos1s = pp.tile([P, NT], f32, tag="pos1s")
            gf0s = pp.tile([P, NT], f32, tag="gf0s")
            gf1s = pp.tile([P, NT], f32, tag="gf1s")
            ngf0s = pp.tile([P, NT], f32, tag="ngf0s")
            w0s = pp.tile([P, NT], f32, tag="w0s")
            w1s = pp.tile([P, NT], f32, tag="w1s")

            eq0s = [pp.tile([P, E], f32, tag=f"eq0_{t}", name=f"eq0_{t}")
                    for t in range(NT)]
            eq1s = [pp.tile([P, E], f32, tag=f"eq1_{t}", name=f"eq1_{t}")
                    for t in range(NT)]
            identh = pp.tile([P, P], fp16, tag="identh")
            # fp16 counts: exact to 2048; values above stay > capacity mask
            ohT = pp.tile([E, N], fp16, tag="ohT")
            cum = ohT  # scan runs in place (chunked, with carry)
            zcol = pp.tile([E, 1], f32, tag="zcol")
            nc.vector.memset(zcol[:], 0.0)
            hT3 = pp.tile([P, DC, NSLOT], fp8, tag="hT3")
            act_sh = [pp.tile([fl, N], fp16, tag=f"actsh_{f0}",
                              name=f"actsh_{f0}") for (f0, fl) in FSH_CH]
            selct3 = pp.tile([P, ST, N], fp8, tag="selct3")
            y3 = pp.tile([P, ST, D], fp8, tag="y3")
            repT = [pp.tile([P, N], fp16, tag=f"repT{i}",
                            name=f"repT{i}") for i in range(4)]
            icols = [pp.tile([P, 1], f32, tag=f"icol{c}",
                             name=f"icol{c}") for c in range(ST)]

            def emit_selct(c):
                s0_ = sp.tile([P, N], fp16, tag="s0", bufs=1,
                              name=f"s0_{c}")
                nc.vector.scalar_tensor_tensor(
                    out=s0_[:], in0=repT[0][:],
                    scalar=icols[c][:, 0:1], in1=repT[2][:],
                    op0=Alu.is_equal, op1=Alu.mult)
                s1_ = sp.tile([P, N], fp16, tag="s1", bufs=1,
                              name=f"s1_{c}")
                nc.vector.scalar_tensor_tensor(
                    out=s1_[:], in0=repT[1][:],
                    scalar=icols[c][:, 0:1], in1=repT[3][:],
                    op0=Alu.is_equal, op1=Alu.mult)
                nc.vector.tensor_add(selct3[:, c, :], s0_[:], s1_[:])

            # ====== gate+up weight streaming pool (reused e0 -> e1) ======
            with tc.tile_pool(name="pwh", bufs=2) as pwh:
                # ============ phase R: routing + shared + dispatch =========
                with tc.tile_pool(name="px", bufs=1) as pxp:
                    xsb = pxp.tile([P, NT, D], fp8, tag="xsb")
                    with tc.tile_pool(name="pSel", bufs=1) as psel:
                        selbf = psel.tile([P, NT, NSLOT], fp8, tag="selbf")
                        with tc.tile_pool(name="pxT", bufs=1) as pxq, \
                             tc.tile_pool(name="pR", bufs=2,
                                          space="PSUM") as pR, \
                             tc.tile_pool(name="pS", bufs=1,
                                          space="PSUM") as pS, \
                             tc.tile_pool(name="pD", bufs=1,
                                          space="PSUM") as pD:
                            xT = pxq.tile([P, DC * N], fp16, tag="xT")

                            def xts(d, a, b):
                                # xT is packed group-major on host:
                                # [P, NG, DC, GT*P] flattened
                                g, off = divmod(a, GT * P)
                                base = (g * DC + d) * GT * P
                                return xT[:, base + off:base + off + b - a]
                            # DMA priority: xT group 0 (router-critical)
                            # -> small consts -> xT group 1 -> x ->
                            # expert-0 weights
                            nc.sync.dma_start(xT[:, 0:DC * GT * P],
                                              xtp[:, 0:DC * GT * P])
                            nc.sync.dma_start(rw_sb[:], rwp)
                            nc.sync.dma_start(rb_sb[:], rbp)
                            nc.sync.dma_start(ident[:], identf)
                            nc.sync.dma_start(iota_e[:], iotaE)
                            nc.sync.dma_start(iota_h[:], iotaS)
                            nc.sync.dma_start(coff_t[:], coff)
                            nc.sync.dma_start(pcol_t[:], pcol)
                            nc.vector.tensor_copy(identh[:], ident[:])
                            nc.sync.dma_start(
                                xT[:, DC * GT * P:2 * DC * GT * P],
                                xtp[:, DC * GT * P:2 * DC * GT * P])
                            nc.sync.dma_start(xsb[:], xp)
                            nc.sync.dma_start(swg_sb[:], swgp)
                            nc.sync.dma_start(swu_sb[:], swup)
                            for (f0, fl), sd_ in zip(FSH_CH, swd_sb):
                                nc.sync.dma_start(sd_[:],
                                                  swdp[f0:f0 + fl, :])
                            wh0 = [pwh.tile([P, DC, HW2], fp8, tag="wh",
                                            name="wh00"),
                                   pwh.tile([P, DC, HW2], fp8, tag="wh",
                                            name="wh01")]
                            nc.sync.dma_start(wh0[0][:], whp[0])
                            nc.sync.dma_start(wh0[1][:], whp[1])

                            def emit_shared(fi, n):
                                f0, fl = FSH_CH[fi]
                                psg = pS.tile([P, 512], f32, space="PSUM",
                                              tag="psg")
                                psu = pS.tile([P, 512], f32, space="PSUM",
                                              tag="psu")
                                for d in range(DC):
                                    nc.tensor.matmul(
                                        psg[:fl, :],
                                        swg_sb[:, d * FSH + f0:
                                               d * FSH + f0 + fl],
                                        xts(d, n * 512, (n + 1) * 512),
                                        start=(d == 0), stop=(d == DC - 1))
                                    nc.tensor.matmul(
                                        psu[:fl, :],
                                        swu_sb[:, d * FSH + f0:
                                               d * FSH + f0 + fl],
                                        xts(d, n * 512, (n + 1) * 512),
                                        start=(d == 0), stop=(d == DC - 1))
                                sga = sp.tile([P, 512], fp16, tag="sga",
                                              bufs=2)
                                nc.scalar.activation(
                                    sga[:fl, :], psg[:fl, :], Act.Silu)
                                nc.vector.tensor_tensor(
                                    out=act_sh[fi][:, n * 512:(n + 1) * 512],
                                    in0=sga[:fl, :], in1=psu[:fl, :],
                                    op=Alu.mult)

                            # ---- routing, streamed by group; shared-expert
                            # chunks interleaved as PE filler while the
                            # vector engine paces top-2/positions ----
                            for g in range(NG):
                                t0g = g * GT
                                for t in range(t0g, t0g + GT):
                                    pt = pR.tile([P, 512], f32, space="PSUM",
                                                 tag="pt")
                                    psl = pt[:, 0:E]
                                    for d in range(DC):
                                        nc.tensor.matmul(
                                            psl,
                                            xts(d, t * P, (t + 1) * P),
                                            rw_sb[:, d * E:(d + 1) * E],
                                            start=(d == 0), stop=False)
                                    nc.tensor.matmul(
                                        psl, ones_row[:], rb_sb[:],
                                        start=False, stop=True)
                                    lg = sp.tile([P, E], f32, tag="lg")
                                    nc.scalar.copy(lg[:], psl)
                                    mx = sp.tile([P, 8], f32, tag="mx")
                                    nc.vector.max(mx[:], lg[:])
                                    mi = sp.tile([P, 8], u32, tag="mi")
                                    nc.vector.max_index(mi[:], mx[:], lg[:])
                                    nc.vector.tensor_tensor(
                                        out=d01s[:, t:t + 1], in0=mx[:, 0:1],
                                        in1=mx[:, 1:2], op=Alu.subtract)
                                    nc.vector.tensor_copy(idx0s[:, t:t + 1],
                                                          mi[:, 0:1])
                                    nc.vector.tensor_copy(idx1s[:, t:t + 1],
                                                          mi[:, 1:2])
                                    nc.vector.tensor_scalar(
                                        out=eq0s[t][:], in0=iota_e[:],
                                        scalar1=idx0s[:, t:t + 1],
                                        scalar2=None, op0=Alu.is_equal)
                                    nc.vector.tensor_scalar(
                                        out=eq1s[t][:], in0=iota_e[:],
                                        scalar1=idx1s[:, t:t + 1],
                                        scalar2=None, op0=Alu.is_equal)
                                    oh = sp.tile([P, E], f32, tag="oh")
                                    nc.vector.tensor_add(oh[:], eq0s[t][:],
                                                         eq1s[t][:])
                                    pso = pt[0:E, 128:256]
                                    nc.tensor.transpose(pso, oh[:], ident[:])
                                    nc.scalar.copy(
                                        ohT[:, t * P:(t + 1) * P], pso)
                                    if (t - t0g) % 2 == 1:
                                        q = (t - t0g) // 2
                                        emit_shared(q // 2, 2 * g + q % 2)

                                ini = (0.0 if g == 0
                                       else cum[:, t0g * P - 1:t0g * P])
                                nc.vector.tensor_tensor_scan(
                                    cum[:, t0g * P:(t0g + GT) * P],
                                    ohT[:, t0g * P:(t0g + GT) * P],
                                    zcol[:, 0:1].to_broadcast([E, GT * P]),
                                    ini, op0=Alu.add, op1=Alu.add)

                                for t in range(t0g, t0g + GT):
                                    pt2 = pR.tile([P, 512], f32, space="PSUM",
                                                  tag="pt")
                                    pcp = pt2[:, 0:E // 2].bitcast(fp16)
                                    nc.tensor.transpose(
                                        pcp, cum[:, t * P:(t + 1) * P],
                                        identh[0:E, 0:E])
                                    cumP = sp.tile([P, E], f32, tag="cumP")
                                    nc.scalar.copy(cumP[:], pcp)
                                    scr = sp.tile([P, E], f32, tag="scr")
                                    nc.vector.tensor_mul(scr[:], eq0s[t][:],
                                                         cumP[:])
                                    nc.vector.reduce_sum(
                                        pos0s[:, t:t + 1], scr[:],
                                        axis=mybir.AxisListType.X)
                                    scr2 = sp.tile([P, E], f32, tag="scr2")
                                    nc.vector.tensor_mul(scr2[:], eq1s[t][:],
                                                         cumP[:])
                                    nc.vector.reduce_sum(
                                        pos1s[:, t:t + 1], scr2[:],
                                        axis=mybir.AxisListType.X)

                                # ---- slot ids + weights for this group ----
                                gs = slice(t0g, t0g + GT)
                                nc.scalar.activation(w0s[:, gs], d01s[:, gs],
                                                     Act.Sigmoid)
                                nc.vector.tensor_scalar(
                                    out=w1s[:, gs], in0=w0s[:, gs],
                                    scalar1=-1.0, scalar2=1.0,
                                    op0=Alu.mult, op1=Alu.add)
                                for (idxs, poss, gfs_) in (
                                        (idx0s, pos0s, gf0s),
                                        (idx1s, pos1s, gf1s)):
                                    loc = sp.tile([P, GT], f32, tag="loc")
                                    nc.vector.tensor_scalar(
                                        out=loc[:], in0=idxs[:, gs],
                                        scalar1=coff_t[:, 0:1], scalar2=None,
                                        op0=Alu.subtract)
                                    pm1 = sp.tile([P, GT], f32, tag="pm1")
                                    nc.vector.tensor_scalar_add(
                                        pm1[:], poss[:, gs], -1.0)
                                    gr = sp.tile([P, GT], f32, tag="gr")
                                    nc.vector.scalar_tensor_tensor(
                                        out=gr[:], in0=loc[:],
                                        scalar=float(C), in1=pm1[:],
                                        op0=Alu.mult, op1=Alu.add)
                                    b1 = sp.tile([P, GT], f32, tag="b1")
                                    nc.vector.tensor_scalar(
                                        out=b1[:], in0=gr[:], scalar1=-0.5,
                                        scalar2=None, op0=Alu.is_gt)
                                    b2 = sp.tile([P, GT], f32, tag="b2")
                                    nc.vector.tensor_scalar(
                                        out=b2[:], in0=gr[:],
                                        scalar1=float(NSLOT) - 0.5,
                                        scalar2=None, op0=Alu.is_lt)
                                    b3 = sp.tile([P, GT], f32, tag="b3")
                                    nc.vector.tensor_scalar(
                                        out=b3[:], in0=pm1[:],
                                        scalar1=float(C) - 0.5,
                                        scalar2=None, op0=Alu.is_lt)
                                    val = sp.tile([P, GT], f32, tag="val")
                                    nc.vector.tensor_mul(val[:], b1[:], b2[:])
                                    nc.vector.tensor_mul(val[:], val[:],
                                                         b3[:])
                                    gm = sp.tile([P, GT], f32, tag="gm")
                                    nc.vector.tensor_scalar_add(
                                        gm[:], gr[:], -float(TRASH))
                                    nc.vector.tensor_mul(gm[:], gm[:], val[:])
                                    nc.vector.tensor_scalar_add(
                                        gfs_[:, gs], gm[:], float(TRASH))
                                nc.vector.tensor_scalar(
                                    out=ngf0s[:, gs], in0=gf0s[:, gs],
                                    scalar1=-1.0, scalar2=None, op0=Alu.mult)

                                # ---- Sel build (token-major 0/1) ----
                                # alternate k0-onehot between scalar engine
                                # (2-pass abs/relu) and vector (is_equal)
                                for t in range(t0g, t0g + GT):
                                    if t % 2 == 0:
                                        ab = sp.tile([P, NSLOT], fp16,
                                                     tag="ab", bufs=2)
                                        nc.scalar.activation(
                                            ab[:], iota_h[:], Act.Abs,
                                            bias=ngf0s[:, t:t + 1], scale=1.0)
                                        sel0 = sp.tile([P, NSLOT], fp16,
                                                       tag="sel0", bufs=2)
                                        nc.scalar.activation(
                                            sel0[:], ab[:], Act.Relu,
                                            bias=1.0, scale=-1.0)
                                    else:
                                        sel0 = sp.tile([P, NSLOT], fp16,
                                                       tag="sel0", bufs=2)
                                        nc.vector.tensor_scalar(
                                            out=sel0[:], in0=iota_h[:],
                                            scalar1=gf0s[:, t:t + 1],
                                            scalar2=None, op0=Alu.is_equal)
                                    nc.vector.scalar_tensor_tensor(
                                        out=selbf[:, t, :], in0=iota_h[:],
                                        scalar=gf1s[:, t:t + 1], in1=sel0[:],
                                        op0=Alu.is_equal, op1=Alu.add)

                            # ---- dispatch: 4 passes (expert, d-half),
                            # fp8 DoubleRow over token-tile pairs ----
                            def emit_dispatch(e, dh):
                                psh = [pD.tile([P, C], f32, space="PSUM",
                                               tag=f"psh{j}",
                                               name=f"psh{e}{dh}{j}")
                                       for j in range(4)]
                                for tp in range(NT // 2):
                                    for j in range(4):
                                        d = dh * 4 + j
                                        nc.tensor.matmul(
                                            psh[j][:],
                                            xsb[:, 2 * tp:2 * tp + 2,
                                                d * P:(d + 1) * P],
                                            selbf[:, 2 * tp:2 * tp + 2,
                                                  e * C:(e + 1) * C],
                                            start=(tp == 0),
                                            stop=(tp == NT // 2 - 1),
                                            perf_mode=DR)
                                for j in range(4):
                                    d = dh * 4 + j
                                    nc.scalar.copy(
                                        hT3[:, d, e * C:(e + 1) * C],
                                        psh[j][:])

                            emit_dispatch(0, 0)

                            # ====== gf/w broadcast rows (PE work lands
                            # between dispatch passes; SelCT STTs are
                            # deferred into the expert phase) ======
                            for c in range(ST):
                                nc.vector.tensor_scalar_add(
                                    icols[c][:], pcol_t[:], float(c * P))
                            for i, src in enumerate((gf0s, gf1s, w0s, w1s)):
                                pgt = pR.tile([P, 512], f32, space="PSUM",
                                              tag="pt")
                                nc.tensor.transpose(pgt[0:NT, 0:P], src[:],
                                                    ident[:])
                                g16 = sp.tile([NT, P], fp16, tag="g16")
                                nc.scalar.copy(g16[:], pgt[0:NT, 0:P])
                                rowb = psel.tile([1, N], fp16, tag="rowb",
                                                 bufs=1, name=f"rowb{i}")
                                # scalar-engine DMA ring: stays clear of
                                # the big weight loads on the sync ring
                                nc.scalar.dma_start(rowb[:], g16[:])
                                for q in range(4):
                                    pgo = pR.tile([P, 512], f32,
                                                  space="PSUM", tag="pt")
                                    nc.tensor.matmul(
                                        pgo[:], ones_row[:],
                                        rowb[0:1, q * 512:(q + 1) * 512],
                                        start=True, stop=True)
                                    nc.vector.tensor_copy(
                                        repT[i][:, q * 512:(q + 1) * 512],
                                        pgo[:])

                            emit_dispatch(0, 1)
                            emit_dispatch(1, 0)
                            emit_dispatch(1, 1)
                        # pxT + psum pools closed (xT freed)
                    # pSel closed (selbf, repT freed)
                # px closed (xsb freed)

                # prefetch expert-1 gate/up (waits on e0 buffer release)
                wh1 = [pwh.tile([P, DC, HW2], fp8, tag="wh", name="wh10"),
                       pwh.tile([P, DC, HW2], fp8, tag="wh", name="wh11")]
                nc.sync.dma_start(wh1[0][:], whp[2])
                nc.sync.dma_start(wh1[1][:], whp[3])
                whs = [wh0, wh1]

                with tc.tile_pool(name="pwd", bufs=2) as pwd:
                    wds = [pwd.tile([P, FC, D], fp8, tag="wdt",
                                    name=f"wd{e}") for e in range(EPC)]
                    nc.sync.dma_start(wds[0][:], wdp[0])
                    nc.sync.dma_start(wds[1][:], wdp[1])

                    # ================= expert MLPs =================
                    pact_cm = tc.tile_pool(name="pact", bufs=16)
                    pact = pact_cm.__enter__()
                    with tc.tile_pool(name="pE", bufs=2,
                                      space="PSUM") as pE, \
                         tc.tile_pool(name="pY", bufs=4,
                                      space="PSUM") as pY:
                        for e in range(EPC):
                            acts3 = pact.tile([P, FC, C], fp8, tag="act3")
                            # fp8 DoubleRow over d-chunk pairs; g/u
                            # alternate two PSUM banks (same-bank
                            # back-to-back runs at half rate)
                            for fi in range(FC):
                                h = 0 if fi < 6 else 1
                                fj = fi - 6 * h
                                fhw = FH[h]
                                psg = pE.tile([P, C], f32, space="PSUM",
                                              tag="psg")
                                psu = pE.tile([P, C], f32, space="PSUM",
                                              tag="psu")
                                for dp in range(DC // 2):
                                    ds = slice(2 * dp, 2 * dp + 2)
                                    go = fj * P
                                    uo = fhw + fj * P
                                    nc.tensor.matmul(
                                        psg[:], whs[e][h][:, ds, go:go + P],
                                        hT3[:, ds, e * C:(e + 1) * C],
                                        start=(dp == 0),
                                        stop=(dp == DC // 2 - 1),
                                        perf_mode=DR)
                                    nc.tensor.matmul(
                                        psu[:], whs[e][h][:, ds, uo:uo + P],
                                        hT3[:, ds, e * C:(e + 1) * C],
                                        start=(dp == 0),
                                        stop=(dp == DC // 2 - 1),
                                        perf_mode=DR)
                                sga = sp.tile([P, C], fp16, tag="esga",
                                              bufs=2)
                                nc.scalar.activation(sga[:], psg[:],
                                                     Act.Silu,
                                                     scale=1.0 / WSC)
                                # acts_dev = 16*act: silu(g) * (64u) / 4;
                                # w_down carries a matching x4 so psy=64*y
                                nc.vector.scalar_tensor_tensor(
                                    out=acts3[:, fi, :], in0=psu[:],
                                    scalar=0.25, in1=sga[:],
                                    op0=Alu.mult, op1=Alu.mult)
                            # deferred SelCT builds (vector) slotted where
                            # they overlap PE down/g-u work
                            if e == 0:
                                emit_selct(0)
                                emit_selct(1)
                            else:
                                emit_selct(3)
                                emit_selct(4)
                            # down-projection -> y tiles (slot-major),
                            # two interleaved PSUM banks
                            groups = [(n, ch) for n in range(2)
                                      for ch in ECH[e]]
                            for gp in range(0, len(groups), 2):
                                pair = groups[gp:gp + 2]
                                psys = [pY.tile([P, 512], f32, space="PSUM",
                                                tag="psy",
                                                name=f"psy{e}_{gp}_{i}")
                                        for i in range(len(pair))]
                                for fp_ in range(6):
                                    fda = slice(2 * fp_, 2 * fp_ + 2)
                                    last = fp_ == 5
                                    for i, (n, (s0_, sl, yc, yr)) in \
                                            enumerate(pair):
                                        # DR requires dst partition base 0;
                                        # the offset-64 chunk runs regular
                                        if yr == 0 and not last:
                                            nc.tensor.matmul(
                                                psys[i][yr:yr + sl, :],
                                                acts3[:, fda, s0_:s0_ + sl],
                                                wds[e][:, fda, n * 512:
                                                       (n + 1) * 512],
                                                start=(fp_ == 0), stop=False,
                                                perf_mode=DR)
                                        elif yr == 0:
                                            nc.tensor.matmul(
                                                psys[i][yr:yr + sl, :],
                                                acts3[:, FC - 1,
                                                      s0_:s0_ + sl],
                                                wds[e][:, FC - 1, n * 512:
                                                       (n + 1) * 512],
                                                start=False, stop=True)
                                        else:
                                            for fo in (0, 1) if not last \
                                                    else (0,):
                                                fi_ = (FC - 1 if last
                                                       else 2 * fp_ + fo)
                                                nc.tensor.matmul(
                                                    psys[i][yr:yr + sl, :],
                                                    acts3[:, fi_,
                                                          s0_:s0_ + sl],
                                                    wds[e][:, fi_, n * 512:
                                                           (n + 1) * 512],
                                                    start=(fp_ == 0
                                                           and fo == 0),
                                                    stop=last)
                                for i, (n, (s0_, sl, yc, yr)) in \
                                        enumerate(pair):
                                    # y3 = 32*y = psy/2 (scalar engine)
                                    nc.scalar.mul(
                                        y3[yr:yr + sl, yc,
                                           n * 512:(n + 1) * 512],
                                        psys[i][yr:yr + sl, :], 0.5)
                            if e == 0:
                                emit_selct(2)
                    pact_cm.__exit__(None, None, None)

                    # ====== combine: out_t = shared_down + SelCT_t^T @ y ====
                    with tc.tile_pool(name="pcx", bufs=3) as pcx, \
                         tc.tile_pool(name="pC", bufs=4, space="PSUM") as pC:
                        for t in range(NT):
                            yout = pcx.tile([P, D], fp16, tag="yout")
                            pscs = [pC.tile([P, 512], f32, space="PSUM",
                                            tag="psc", name=f"psc{t}_{n}")
                                    for n in range(2)]
                            for fi, (f0, fl) in enumerate(FSH_CH):
                                for n in range(2):
                                    nc.tensor.matmul(
                                        pscs[n][:],
                                        act_sh[fi][:, t * P:(t + 1) * P],
                                        swd_sb[fi][:, n * 512:(n + 1) * 512],
                                        start=(fi == 0), stop=False)
                            for cp_ in range(3):
                                ca = slice(2 * cp_, 2 * cp_ + 2)
                                last = cp_ == 2
                                for n in range(2):
                                    if not last:
                                        nc.tensor.matmul(
                                            pscs[n][:],
                                            selct3[:, ca, t * P:(t + 1) * P],
                                            y3[:, ca, n * 512:(n + 1) * 512],
                                            start=False, stop=False,
                                            perf_mode=DR)
                                    else:
                                        nc.tensor.matmul(
                                            pscs[n][:],
                                            selct3[:, ST - 1,
                                                   t * P:(t + 1) * P],
                                            y3[:, ST - 1,
                                               n * 512:(n + 1) * 512],
                                            start=False, stop=True)
                            for n in range(2):
                                # yout = psc/32 (shared swd carries x32)
                                nc.scalar.mul(
                                    yout[:, n * 512:(n + 1) * 512],
                                    pscs[n][:], 1.0 / 32.0)
                            nc.sync.dma_start(
                                partial[t * P:(t + 1) * P, :], yout[:])

    nc.compile()
    return nc


def _get_nc():
    if "nc" not in _BUILD_CACHE:
        _BUILD_CACHE["nc"] = _build_nc()
    return _BUILD_CACHE["nc"]


def _pack_dchunk(a, width):
    """[D, W] -> [128, DC*W] with d-chunk-major free layout."""
    d, w = a.shape
    return np.ascontiguousarray(
        a.reshape(d // P, P, w).transpose(1, 0, 2).reshape(P, (d // P) * w))


def kernel(x, router_w, router_b, w_gate, w_up, w_down, sw_gate, sw_up,
           sw_down, _trace=False):
    import ml_dtypes
    f16 = np.float16
    f8 = np.dtype(ml_dtypes.float8_e4m3)
    from concourse.bass_utils import run_bass_kernel_spmd

    x = np.asarray(x, np.float32)
    x2 = np.ascontiguousarray(x.reshape(N, D))
    xp = np.ascontiguousarray(
        x2.reshape(NT, P, D).transpose(1, 0, 2)).astype(f8)
    xT_ = np.ascontiguousarray(x2.T)  # [D, N]
    # group-major packing: [P, NG, DC, GT*P]
    xtp = np.ascontiguousarray(
        xT_.reshape(DC, P, NG, GT * P).transpose(1, 2, 0, 3)
        .reshape(P, DC * N)).astype(f16)
    rwp = _pack_dchunk(np.asarray(router_w, np.float32), E).astype(f16)
    rbp = np.asarray(router_b, np.float32).reshape(1, E).astype(f16)
    wg = np.asarray(w_gate, np.float32)
    wu = np.asarray(w_up, np.float32)
    wdn = np.asarray(w_down, np.float32) * 4.0  # acts carry 16x, psy=64y
    swg = np.asarray(sw_gate, np.float32)
    swu = np.asarray(sw_up, np.float32)
    swd = np.asarray(sw_down, np.float32)

    iotaE = np.broadcast_to(np.arange(E, dtype=np.float32), (P, E)).copy()
    iotaS = np.broadcast_to(np.arange(NSLOT, dtype=f16), (P, NSLOT)).copy()
    identf = np.eye(P, dtype=np.float32)
    pcol = np.arange(P, dtype=np.float32).reshape(P, 1)

    in_maps = []
    for m in range(NCORES):
        fs = slice(m * FSH, (m + 1) * FSH)
        wh_list = []
        for e in range(EPC):
            ge = wg[m * EPC + e] * WSC
            ue = wu[m * EPC + e] * WSC
            for (a, b) in ((0, 768), (768, F)):
                fhw = b - a
                cat = np.concatenate([ge[:, a:b], ue[:, a:b]], axis=1)
                # [D, 2fhw] -> [128, DC, HW2] (per-d-chunk, padded rows)
                p3 = cat.reshape(DC, P, 2 * fhw).transpose(1, 0, 2)
                if 2 * fhw < HW2:
                    p3 = np.pad(p3, ((0, 0), (0, 0), (0, HW2 - 2 * fhw)))
                wh_list.append(p3)
        whp_ = np.ascontiguousarray(np.stack(wh_list)).astype(f8)
        wdp_ = np.ascontiguousarray(np.stack(
            [wdn[m * EPC + e].reshape(FC, P, D).transpose(1, 0, 2)
             for e in range(EPC)])).astype(f8)
        in_maps.append({
            "xp": xp,
            "xtp": xtp,
            "rwp": rwp,
            "rbp": rbp,
            "whp": whp_,
            "wdp": wdp_,
            "swgp": _pack_dchunk(swg[:, fs], FSH).astype(f16),
            "swup": _pack_dchunk(swu[:, fs], FSH).astype(f16),
            "swdp": np.ascontiguousarray(swd[fs, :] * 32.0).astype(f16),
            "coff": np.full((P, 1), float(m * EPC), np.float32),
            "pcol": pcol,
            "iotaE": iotaE,
            "iotaS": iotaS,
            "identf": identf,
        })

    nc = _get_nc()
    res = run_bass_kernel_spmd(nc, in_maps, core_ids=list(range(NCORES)),
                               trace=_trace)
    out = x2.copy()
    for r in res.results:
        out += r["partial"].astype(np.float32)
    if _trace:
        kernel._last_results = res
    return out.reshape(x.shape)


# revision 53
# speedup vs baseline: 1.0390x; 1.0390x over previous
"""DeepseekMoE Trainium2 Bass kernel (8-core expert-parallel, v5).

kernel(**inputs) takes FULL unsharded inputs (as produced by setup_inputs)
and returns the FULL output [1, 2048, 1024] fp32.

Sharding (8 cores):
  - Expert-parallel: 2 of 16 experts per core.
  - Shared expert: F-dim sliced 1408/8=176 per core (partial sums).
  - Router replicated per core.
  - Host: out = x + sum(per-core partials).

v5 design:
  - fp16 compute, fp8(e4m3) DoubleRow matmuls for dispatch and expert
    gate/up (weights pre-scaled x64 on host; the descale rides through
    silu's scale arg and a 1/64 fold into w_down).
  - Zero gpsimd usage (its tensor ops are ~20x slow AND stall the vector
    engine); iotas/identity are host inputs.
  - Sel (token-major 0/1) built per tile with scalar-engine 2-pass
    relu(1-|iota-gf0|) (even tiles) or vector is_equal (odd tiles),
    + 1 vector STT ((iota==gf1)+sel0).
  - SelCT (slot-major, prob-weighted) built directly via STT with the
    per-partition slot id as scalar, against PE-broadcast gf/w rows;
    scheduled inside the dispatch window, off the critical path.
  - Routing streamed in 2 groups of 8 token tiles (chunked in-place scan).
  - Dispatch as 4 passes (expert x d-half), 4 PSUM banks each, fp8
    DoubleRow over token-tile pairs, streaming behind Sel production.
  - Expert weights host-packed fp8: one DMA descriptor per (expert,
    F-half) for gate+up, one fp16 per expert for down; streamed through
    2-buffer pools.
  - Partial output in fp16.
"""
import numpy as np

# ---- problem constants (hardcoded; kernel.py must be self-contained) ----
N = 2048          # tokens
D = 1024          # model dim
E = 16            # experts
F = 1408          # expert ffn dim
C = 320           # per-expert capacity = ceil(1.25 * N*K / E)
NCORES = 8
EPC = E // NCORES  # experts per core = 2
FSH = F // NCORES  # shared-expert F slice = 176
P = 128
NT = N // P        # 16 token tiles
DC = D // P        # 8 d-chunks
FC = F // P        # 11 f-chunks
NSLOT = EPC * C    # 640 slots per core
TRASH = NSLOT      # sentinel slot id (matches nothing in iota 0..639)
ST = NSLOT // P    # 5 slot tiles
NG = 2             # routing groups
GT = NT // NG      # tiles per group = 8
FH = (768, 640)    # expert F halves (6 + 5 chunks of 128)
HW2 = 2 * 768      # per-d-chunk packed row width (h1 padded to match)
WSC = 64.0         # fp8 weight pre-scale (keeps w in e4m3 normal range)

_BUILD_CACHE = {}


def _build_nc():
    import concourse.bacc as bacc
    import concourse.mybir as mybir
    import concourse.tile as tile

    f32 = mybir.dt.float32
    fp16 = mybir.dt.float16
    fp8 = mybir.dt.float8e4
    u32 = mybir.dt.uint32
    Alu = mybir.AluOpType
    Act = mybir.ActivationFunctionType
    DR = mybir.MatmulPerfMode.DoubleRow

    nc = bacc.Bacc("TRN2", target_bir_lowering=False, debug=False)

    # ---- I/O (all host-packed) ----
    xp = nc.dram_tensor("xp", [P, NT, D], fp8, kind="ExternalInput").ap()
    xtp = nc.dram_tensor("xtp", [P, DC * N], fp16, kind="ExternalInput").ap()
    rwp = nc.dram_tensor("rwp", [P, DC * E], fp16, kind="ExternalInput").ap()
    rbp = nc.dram_tensor("rbp", [1, E], fp16, kind="ExternalInput").ap()
    whp = nc.dram_tensor("whp", [EPC * 2, P, DC, HW2], fp8,
                         kind="ExternalInput").ap()
    wdp = nc.dram_tensor("wdp", [EPC, P, FC, D], fp8,
                         kind="ExternalInput").ap()
    swgp = nc.dram_tensor("swgp", [P, DC * FSH], fp16,
                          kind="ExternalInput").ap()
    swup = nc.dram_tensor("swup", [P, DC * FSH], fp16,
                          kind="ExternalInput").ap()
    swdp = nc.dram_tensor("swdp", [FSH, D], fp16, kind="ExternalInput").ap()
    coff = nc.dram_tensor("coff", [P, 1], f32, kind="ExternalInput").ap()
    pcol = nc.dram_tensor("pcol", [P, 1], f32, kind="ExternalInput").ap()
    iotaE = nc.dram_tensor("iotaE", [P, E], f32, kind="ExternalInput").ap()
    iotaS = nc.dram_tensor("iotaS", [P, NSLOT], fp16,
                           kind="ExternalInput").ap()
    identf = nc.dram_tensor("identf", [P, P], f32, kind="ExternalInput").ap()
    partial = nc.dram_tensor("partial", [N, D], fp16,
                             kind="ExternalOutput").ap()

    FSH_CH = [(0, P), (P, FSH - P)]        # shared-expert f chunks: 128 + 48
    # per-expert slot sub-chunks (offset within expert, rows) and the
    # global y-tile/row they land in
    ECH = {0: [(0, 128, 0, 0), (128, 128, 1, 0), (256, 64, 2, 0)],
           1: [(0, 64, 2, 64), (64, 128, 3, 0), (192, 128, 4, 0)]}

    with tile.TileContext(nc) as tc:
        with tc.tile_pool(name="persist", bufs=1) as pp, \
             tc.tile_pool(name="stream", bufs=4) as sp:

            # ---- tiles for constants (DMAs issued after xT below) ----
            ident = pp.tile([P, P], f32, tag="ident")
            iota_e = pp.tile([P, E], f32, tag="iota_e")
            iota_h = pp.tile([P, NSLOT], fp16, tag="iota_h")
            coff_t = pp.tile([P, 1], f32, tag="coff_t")
            pcol_t = pp.tile([P, 1], f32, tag="pcol_t")
            rw_sb = pp.tile([P, DC * E], fp16, tag="rw_sb")
            rb_sb = pp.tile([1, E], fp16, tag="rb_sb")
            ones_row = pp.tile([1, P], fp16, tag="ones_row")
            nc.vector.memset(ones_row[:], 1.0)
            swg_sb = pp.tile([P, DC * FSH], fp16, tag="swg_sb")
            swu_sb = pp.tile([P, DC * FSH], fp16, tag="swu_sb")
            swd_sb = [pp.tile([fl, D], fp16, tag=f"swd_{f0}",
                              name=f"swd_{f0}") for (f0, fl) in FSH_CH]

            # routing staging [128, NT] (column = token tile), f32
            d01s = pp.tile([P, NT], f32, tag="d01s")
            idx0s = pp.tile([P, NT], f32, tag="idx0s")
            idx1s = pp.tile([P, NT], f32, tag="idx1s")
            pos0s = pp.tile([P, NT], f32, tag="pos0s")
            pos1s = pp.tile([P, NT], f32, tag="pos1s")
            gf0s = pp.tile([P, NT], f32, tag="gf0s")
            gf1s = pp.tile([P, NT], f32, tag="gf1s")
            ngf0s = pp.tile([P, NT], f32, tag="ngf0s")
            w0s = pp.tile([P, NT], f32, tag="w0s")
            w1s = pp.tile([P, NT], f32, tag="w1s")

            eq0s = [pp.tile([P, E], f32, tag=f"eq0_{t}", name=f"eq0_{t}")
                    for t in range(NT)]
            eq1s = [pp.tile([P, E], f32, tag=f"eq1_{t}", name=f"eq1_{t}")
                    for t in range(NT)]
            identh = pp.tile([P, P], fp16, tag="identh")
            # fp16 counts: exact to 2048; values above stay > capacity mask
            ohT = pp.tile([E, N], fp16, tag="ohT")
            cum = ohT  # scan runs in place (chunked, with carry)
            zcol = pp.tile([E, 1], f32, tag="zcol")
            nc.vector.memset(zcol[:], 0.0)
            hT3 = pp.tile([P, DC, NSLOT], fp8, tag="hT3")
            act_sh = [pp.tile([fl, N], fp16, tag=f"actsh_{f0}",
                              name=f"actsh_{f0}") for (f0, fl) in FSH_CH]
            selct3 = pp.tile([P, ST, N], fp8, tag="selct3")
            y3 = pp.tile([P, ST, D], fp8, tag="y3")
            repT = [pp.tile([P, N], fp16, tag=f"repT{i}",
                            name=f"repT{i}") for i in range(4)]
            icols = [pp.tile([P, 1], f32, tag=f"icol{c}",
                             name=f"icol{c}") for c in range(ST)]

            def emit_selct(c):
                s0_ = sp.tile([P, N], fp16, tag="s0", bufs=1,
                              name=f"s0_{c}")
                nc.vector.scalar_tensor_tensor(
                    out=s0_[:], in0=repT[0][:],
                    scalar=icols[c][:, 0:1], in1=repT[2][:],
                    op0=Alu.is_equal, op1=Alu.mult)
                s1_ = sp.tile([P, N], fp16, tag="s1", bufs=1,
                              name=f"s1_{c}")
                nc.vector.scalar_tensor_tensor(
                    out=s1_[:], in0=repT[1][:],
                    scalar=icols[c][:, 0:1], in1=repT[3][:],
                    op0=Alu.is_equal, op1=Alu.mult)
                nc.vector.tensor_add(selct3[:, c, :], s0_[:], s1_[:])

            # ====== gate+up weight streaming pool (reused e0 -> e1) ======
            with tc.tile_pool(name="pwh", bufs=2) as pwh:
                # ============ phase R: routing + shared + dispatch =========
                with tc.tile_pool(name="px", bufs=1) as pxp:
                    xsb = pxp.tile([P, NT, D], fp8, tag="xsb")
                    with tc.tile_pool(name="pSel", bufs=1) as psel:
                        selbf = psel.tile([P, NT, NSLOT], fp8, tag="selbf")
                        with tc.tile_pool(name="pxT", bufs=1) as pxq, \
                             tc.tile_pool(name="pR", bufs=2,
                                          space="PSUM") as pR, \
                             tc.tile_pool(name="pS", bufs=1,
                                          space="PSUM") as pS, \
                             tc.tile_pool(name="pD", bufs=1,
                                          space="PSUM") as pD:
                            xT = pxq.tile([P, DC * N], fp16, tag="xT")

                            def xts(d, a, b):
                                # xT is packed group-major on host:
                                # [P, NG, DC, GT*P] flattened
                                g, off = divmod(a, GT * P)
                                base = (g * DC + d) * GT * P
                                return xT[:, base + off:base + off + b - a]
                            # DMA priority: xT group 0 (router-critical)
                            # -> small consts -> xT group 1 -> x ->
                            # expert-0 weights
                            nc.sync.dma_start(xT[:, 0:DC * GT * P],
                                              xtp[:, 0:DC * GT * P])
                            nc.sync.dma_start(rw_sb[:], rwp)
                            nc.sync.dma_start(rb_sb[:], rbp)
                            nc.sync.dma_start(ident[:], identf)
                            nc.sync.dma_start(iota_e[:], iotaE)
                            nc.sync.dma_start(iota_h[:], iotaS)
                            nc.sync.dma_start(coff_t[:], coff)
                            nc.sync.dma_start(pcol_t[:], pcol)
                            nc.vector.tensor_copy(identh[:], ident[:])
                            nc.sync.dma_start(
                                xT[:, DC * GT * P:2 * DC * GT * P],
                                xtp[:, DC * GT * P:2 * DC * GT * P])
                            nc.sync.dma_start(xsb[:], xp)
                            nc.sync.dma_start(swg_sb[:], swgp)
                            nc.sync.dma_start(swu_sb[:], swup)
                            for (f0, fl), sd_ in zip(FSH_CH, swd_sb):
                                nc.sync.dma_start(sd_[:],
                                                  swdp[f0:f0 + fl, :])
                            wh0 = [pwh.tile([P, DC, HW2], fp8, tag="wh",
                                            name="wh00"),
                                   pwh.tile([P, DC, HW2], fp8, tag="wh",
                                            name="wh01")]
                            nc.sync.dma_start(wh0[0][:], whp[0])
                            nc.sync.dma_start(wh0[1][:], whp[1])

                            def emit_shared(fi, n):
                                f0, fl = FSH_CH[fi]
                                psg = pS.tile([P, 512], f32, space="PSUM",
                                              tag="psg")
                                psu = pS.tile([P, 512], f32, space="PSUM",
                                              tag="psu")
                                for d in range(DC):
                                    nc.tensor.matmul(
                                        psg[:fl, :],
                                        swg_sb[:, d * FSH + f0:
                                               d * FSH + f0 + fl],
                                        xts(d, n * 512, (n + 1) * 512),
                                        start=(d == 0), stop=(d == DC - 1))
                                    nc.tensor.matmul(
                                        psu[:fl, :],
                                        swu_sb[:, d * FSH + f0:
                                               d * FSH + f0 + fl],
                                        xts(d, n * 512, (n + 1) * 512),
                                        start=(d == 0), stop=(d == DC - 1))
                                sga = sp.tile([P, 512], fp16, tag="sga",
                                              bufs=2)
                                nc.scalar.activation(
                                    sga[:fl, :], psg[:fl, :], Act.Silu)
                                nc.vector.tensor_tensor(
                                    out=act_sh[fi][:, n * 512:(n + 1) * 512],
                                    in0=sga[:fl, :], in1=psu[:fl, :],
                                    op=Alu.mult)

                            # ---- routing, streamed by group; shared-expert
                            # chunks interleaved as PE filler while the
                            # vector engine paces top-2/positions ----
                            for g in range(NG):
                                t0g = g * GT
                                for t in range(t0g, t0g + GT):
                                    pt = pR.tile([P, 512], f32, space="PSUM",
                                                 tag="pt")
                                    psl = pt[:, 0:E]
                                    for d in range(DC):
                                        nc.tensor.matmul(
                                            psl,
                                            xts(d, t * P, (t + 1) * P),
                                            rw_sb[:, d * E:(d + 1) * E],
                                            start=(d == 0), stop=False)
                                    nc.tensor.matmul(
                                        psl, ones_row[:], rb_sb[:],
                                        start=False, stop=True)
                                    lg = sp.tile([P, E], f32, tag="lg")
                                    nc.scalar.copy(lg[:], psl)
                                    mx = sp.tile([P, 8], f32, tag="mx")
                                    nc.vector.max(mx[:], lg[:])
                                    mi = sp.tile([P, 8], u32, tag="mi")
                                    nc.vector.max_index(mi[:], mx[:], lg[:])
                                    nc.vector.tensor_tensor(
                                        out=d01s[:, t:t + 1], in0=mx[:, 0:1],
                                        in1=mx[:, 1:2], op=Alu.subtract)
                                    nc.vector.tensor_copy(idx0s[:, t:t + 1],
                                                          mi[:, 0:1])
                                    nc.vector.tensor_copy(idx1s[:, t:t + 1],
                                                          mi[:, 1:2])
                                    nc.vector.tensor_scalar(
                                        out=eq0s[t][:], in0=iota_e[:],
                                        scalar1=idx0s[:, t:t + 1],
                                        scalar2=None, op0=Alu.is_equal)
                                    nc.vector.tensor_scalar(
                                        out=eq1s[t][:], in0=iota_e[:],
                                        scalar1=idx1s[:, t:t + 1],
                                        scalar2=None, op0=Alu.is_equal)
                                    oh = sp.tile([P, E], f32, tag="oh")
                                    nc.vector.tensor_add(oh[:], eq0s[t][:],
                                                         eq1s[t][:])
                                    pso = pt[0:E, 128:256]
                                    nc.tensor.transpose(pso, oh[:], ident[:])
                                    nc.scalar.copy(
                                        ohT[:, t * P:(t + 1) * P], pso)

                                ini = (0.0 if g == 0
                                       else cum[:, t0g * P - 1:t0g * P])
                                nc.vector.tensor_tensor_scan(
                                    cum[:, t0g * P:(t0g + GT) * P],
                                    ohT[:, t0g * P:(t0g + GT) * P],
                                    zcol[:, 0:1].to_broadcast([E, GT * P]),
                                    ini, op0=Alu.add, op1=Alu.add)

                                for t in range(t0g, t0g + GT):
                                    pt2 = pR.tile([P, 512], f32, space="PSUM",
                                                  tag="pt")
                                    pcp = pt2[:, 0:E // 2].bitcast(fp16)
                                    nc.tensor.transpose(
                                        pcp, cum[:, t * P:(t + 1) * P],
                                        identh[0:E, 0:E])
                                    cumP = sp.tile([P, E], f32, tag="cumP")
                                    nc.scalar.copy(cumP[:], pcp)
                                    scr = sp.tile([P, E], f32, tag="scr")
                                    nc.vector.tensor_mul(scr[:], eq0s[t][:],
                                                         cumP[:])
                                    nc.vector.reduce_sum(
                                        pos0s[:, t:t + 1], scr[:],
                                        axis=mybir.AxisListType.X)
                                    scr2 = sp.tile([P, E], f32, tag="scr2")
                                    nc.vector.tensor_mul(scr2[:], eq1s[t][:],
                                                         cumP[:])
                                    nc.vector.reduce_sum(
                                        pos1s[:, t:t + 1], scr2[:],
                                        axis=mybir.AxisListType.X)

                                # ---- slot ids + weights for this group ----
                                gs = slice(t0g, t0g + GT)
                                nc.scalar.activation(w0s[:, gs], d01s[:, gs],
                                                     Act.Sigmoid)
                                nc.vector.tensor_scalar(
                                    out=w1s[:, gs], in0=w0s[:, gs],
                                    scalar1=-1.0, scalar2=1.0,
                                    op0=Alu.mult, op1=Alu.add)
                                for (idxs, poss, gfs_) in (
                                        (idx0s, pos0s, gf0s),
                                        (idx1s, pos1s, gf1s)):
                                    loc = sp.tile([P, GT], f32, tag="loc")
                                    nc.vector.tensor_scalar(
                                        out=loc[:], in0=idxs[:, gs],
                                        scalar1=coff_t[:, 0:1], scalar2=None,
                                        op0=Alu.subtract)
                                    pm1 = sp.tile([P, GT], f32, tag="pm1")
                                    nc.vector.tensor_scalar_add(
                                        pm1[:], poss[:, gs], -1.0)
                                    gr = sp.tile([P, GT], f32, tag="gr")
                                    nc.vector.scalar_tensor_tensor(
                                        out=gr[:], in0=loc[:],
                                        scalar=float(C), in1=pm1[:],
                                        op0=Alu.mult, op1=Alu.add)
                                    b1 = sp.tile([P, GT], f32, tag="b1")
                                    nc.vector.tensor_scalar(
                                        out=b1[:], in0=gr[:], scalar1=-0.5,
                                        scalar2=None, op0=Alu.is_gt)
                                    b2 = sp.tile([P, GT], f32, tag="b2")
                                    nc.vector.tensor_scalar(
                                        out=b2[:], in0=gr[:],
                                        scalar1=float(NSLOT) - 0.5,
                                        scalar2=None, op0=Alu.is_lt)
                                    b3 = sp.tile([P, GT], f32, tag="b3")
                                    nc.vector.tensor_scalar(
                                        out=b3[:], in0=pm1[:],
                                        scalar1=float(C) - 0.5,
                                        scalar2=None, op0=Alu.is_lt)
                                    val = sp.tile([P, GT], f32, tag="val")
                                    nc.vector.tensor_mul(val[:], b1[:], b2[:])
                                    nc.vector.tensor_mul(val[:], val[:],
                                                         b3[:])
                                    gm = sp.tile([P, GT], f32, tag="gm")
                                    nc.vector.tensor_scalar_add(
                                        gm[:], gr[:], -float(TRASH))
                                    nc.vector.tensor_mul(gm[:], gm[:], val[:])
                                    nc.vector.tensor_scalar_add(
                                        gfs_[:, gs], gm[:], float(TRASH))
                                nc.vector.tensor_scalar(
                                    out=ngf0s[:, gs], in0=gf0s[:, gs],
                                    scalar1=-1.0, scalar2=None, op0=Alu.mult)

                                # ---- Sel build (token-major 0/1) ----
                                # alternate k0-onehot between scalar engine
                                # (2-pass abs/relu) and vector (is_equal)
                                for t in range(t0g, t0g + GT):
                                    if t % 2 == 0:
                                        ab = sp.tile([P, NSLOT], fp16,
                                                     tag="ab", bufs=2)
                                        nc.scalar.activation(
                                            ab[:], iota_h[:], Act.Abs,
                                            bias=ngf0s[:, t:t + 1], scale=1.0)
                                        sel0 = sp.tile([P, NSLOT], fp16,
                                                       tag="sel0", bufs=2)
                                        nc.scalar.activation(
                                            sel0[:], ab[:], Act.Relu,
                                            bias=1.0, scale=-1.0)
                                    else:
                                        sel0 = sp.tile([P, NSLOT], fp16,
                                                       tag="sel0", bufs=2)
                                        nc.vector.tensor_scalar(
                                            out=sel0[:], in0=iota_h[:],
                                            scalar1=gf0s[:, t:t + 1],
                                            scalar2=None, op0=Alu.is_equal)
                                    nc.vector.scalar_tensor_tensor(
                                        out=selbf[:, t, :], in0=iota_h[:],
                                        scalar=gf1s[:, t:t + 1], in1=sel0[:],
                                        op0=Alu.is_equal, op1=Alu.add)

                            # ---- shared expert gate/up (PE filler) ----
                            for fi in range(2):
                                for n in range(4):
                                    emit_shared(fi, n)

                            # ---- dispatch: 4 passes (expert, d-half),
                            # fp8 DoubleRow over token-tile pairs ----
                            def emit_dispatch(e, dh):
                                psh = [pD.tile([P, C], f32, space="PSUM",
                                               tag=f"psh{j}",
                                               name=f"psh{e}{dh}{j}")
                                       for j in range(4)]
                                for tp in range(NT // 2):
                                    for j in range(4):
                                        d = dh * 4 + j
                                        nc.tensor.matmul(
                                            psh[j][:],
                                            xsb[:, 2 * tp:2 * tp + 2,
                                                d * P:(d + 1) * P],
                                            selbf[:, 2 * tp:2 * tp + 2,
                                                  e * C:(e + 1) * C],
                                            start=(tp == 0),
                                            stop=(tp == NT // 2 - 1),
                                            perf_mode=DR)
                                for j in range(4):
                                    d = dh * 4 + j
                                    nc.scalar.copy(
                                        hT3[:, d, e * C:(e + 1) * C],
                                        psh[j][:])

                            emit_dispatch(0, 0)

                            # ====== gf/w broadcast rows (PE work lands
                            # between dispatch passes; SelCT STTs are
                            # deferred into the expert phase) ======
                            for c in range(ST):
                                nc.vector.tensor_scalar_add(
                                    icols[c][:], pcol_t[:], float(c * P))
                            for i, src in enumerate((gf0s, gf1s, w0s, w1s)):
                                pgt = pR.tile([P, 512], f32, space="PSUM",
                                              tag="pt")
                                nc.tensor.transpose(pgt[0:NT, 0:P], src[:],
                                                    ident[:])
                                g16 = sp.tile([NT, P], fp16, tag="g16")
                                nc.scalar.copy(g16[:], pgt[0:NT, 0:P])
                                rowb = psel.tile([1, N], fp16, tag="rowb",
                                                 bufs=1, name=f"rowb{i}")
                                # scalar-engine DMA ring: stays clear of
                                # the big weight loads on the sync ring
                                nc.scalar.dma_start(rowb[:], g16[:])
                                for q in range(4):
                                    pgo = pR.tile([P, 512], f32,
                                                  space="PSUM", tag="pt")
                                    nc.tensor.matmul(
                                        pgo[:], ones_row[:],
                                        rowb[0:1, q * 512:(q + 1) * 512],
                                        start=True, stop=True)
                                    nc.vector.tensor_copy(
                                        repT[i][:, q * 512:(q + 1) * 512],
                                        pgo[:])

                            emit_dispatch(0, 1)
                            emit_dispatch(1, 0)
                            emit_dispatch(1, 1)
                        # pxT + psum pools closed (xT freed)
                    # pSel closed (selbf, repT freed)
                # px closed (xsb freed)

                # prefetch expert-1 gate/up (waits on e0 buffer release)
                wh1 = [pwh.tile([P, DC, HW2], fp8, tag="wh", name="wh10"),
                       pwh.tile([P, DC, HW2], fp8, tag="wh", name="wh11")]
                nc.sync.dma_start(wh1[0][:], whp[2])
                nc.sync.dma_start(wh1[1][:], whp[3])
                whs = [wh0, wh1]

                with tc.tile_pool(name="pwd", bufs=2) as pwd:
                    wds = [pwd.tile([P, FC, D], fp8, tag="wdt",
                                    name=f"wd{e}") for e in range(EPC)]
                    nc.sync.dma_start(wds[0][:], wdp[0])
                    nc.sync.dma_start(wds[1][:], wdp[1])

                    # ================= expert MLPs =================
                    pact_cm = tc.tile_pool(name="pact", bufs=16)
                    pact = pact_cm.__enter__()
                    with tc.tile_pool(name="pE", bufs=2,
                                      space="PSUM") as pE, \
                         tc.tile_pool(name="pY", bufs=4,
                                      space="PSUM") as pY:
                        for e in range(EPC):
                            acts3 = pact.tile([P, FC, C], fp8, tag="act3")
                            # fp8 DoubleRow over d-chunk pairs; g/u
                            # alternate two PSUM banks (same-bank
                            # back-to-back runs at half rate)
                            for fi in range(FC):
                                h = 0 if fi < 6 else 1
                                fj = fi - 6 * h
                                fhw = FH[h]
                                psg = pE.tile([P, C], f32, space="PSUM",
                                              tag="psg")
                                psu = pE.tile([P, C], f32, space="PSUM",
                                              tag="psu")
                                for dp in range(DC // 2):
                                    ds = slice(2 * dp, 2 * dp + 2)
                                    go = fj * P
                                    uo = fhw + fj * P
                                    nc.tensor.matmul(
                                        psg[:], whs[e][h][:, ds, go:go + P],
                                        hT3[:, ds, e * C:(e + 1) * C],
                                        start=(dp == 0),
                                        stop=(dp == DC // 2 - 1),
                                        perf_mode=DR)
                                    nc.tensor.matmul(
                                        psu[:], whs[e][h][:, ds, uo:uo + P],
                                        hT3[:, ds, e * C:(e + 1) * C],
                                        start=(dp == 0),
                                        stop=(dp == DC // 2 - 1),
                                        perf_mode=DR)
                                sga = sp.tile([P, C], fp16, tag="esga",
                                              bufs=2)
                                nc.scalar.activation(sga[:], psg[:],
                                                     Act.Silu,
                                                     scale=1.0 / WSC)
                                # acts_dev = 16*act: silu(g) * (64u) / 4;
                                # w_down carries a matching x4 so psy=64*y
                                nc.vector.scalar_tensor_tensor(
                                    out=acts3[:, fi, :], in0=psu[:],
                                    scalar=0.25, in1=sga[:],
                                    op0=Alu.mult, op1=Alu.mult)
                            # deferred SelCT builds (vector) slotted where
                            # they overlap PE down/g-u work
                            if e == 0:
                                emit_selct(0)
                                emit_selct(1)
                            else:
                                emit_selct(3)
                                emit_selct(4)
                            # down-projection -> y tiles (slot-major),
                            # two interleaved PSUM banks
                            groups = [(n, ch) for n in range(2)
                                      for ch in ECH[e]]
                            for gp in range(0, len(groups), 2):
                                pair = groups[gp:gp + 2]
                                psys = [pY.tile([P, 512], f32, space="PSUM",
                                                tag="psy",
                                                name=f"psy{e}_{gp}_{i}")
                                        for i in range(len(pair))]
                                for fp_ in range(6):
                                    fda = slice(2 * fp_, 2 * fp_ + 2)
                                    last = fp_ == 5
                                    for i, (n, (s0_, sl, yc, yr)) in \
                                            enumerate(pair):
                                        # DR requires dst partition base 0;
                                        # the offset-64 chunk runs regular
                                        if yr == 0 and not last:
                                            nc.tensor.matmul(
                                                psys[i][yr:yr + sl, :],
                                                acts3[:, fda, s0_:s0_ + sl],
                                                wds[e][:, fda, n * 512:
                                                       (n + 1) * 512],
                                                start=(fp_ == 0), stop=False,
                                                perf_mode=DR)
                                        elif yr == 0:
                                            nc.tensor.matmul(
                                                psys[i][yr:yr + sl, :],
                                                acts3[:, FC - 1,
                                                      s0_:s0_ + sl],
                                                wds[e][:, FC - 1, n * 512:
                                                       (n + 1) * 512],
                                                start=False, stop=True)
                                        else:
                                            for fo in (0, 1) if not last \
                                                    else (0,):
                                                fi_ = (FC - 1 if last
                                                       else 2 * fp_ + fo)
                                                nc.tensor.matmul(
                                                    psys[i][yr:yr + sl, :],
                                                    acts3[:, fi_,
                                                          s0_:s0_ + sl],
                                                    wds[e][:, fi_, n * 512:
                                                           (n + 1) * 512],
                                                    start=(fp_ == 0
                                                           and fo == 0),
                                                    stop=last)
                                for i, (n, (s0_, sl, yc, yr)) in \
                                        enumerate(pair):
                                    # y3 = 32*y = psy/2 (scalar engine)
                                    nc.scalar.mul(
                                        y3[yr:yr + sl, yc,
                                           n * 512:(n + 1) * 512],
                                        psys[i][yr:yr + sl, :], 0.5)
                            if e == 0:
                                emit_selct(2)
                    pact_cm.__exit__(None, None, None)

                    # ====== combine: out_t = shared_down + SelCT_t^T @ y ====
                    with tc.tile_pool(name="pcx", bufs=3) as pcx, \
                         tc.tile_pool(name="pC", bufs=4, space="PSUM") as pC:
                        for t in range(NT):
                            yout = pcx.tile([P, D], fp16, tag="yout")
                            pscs = [pC.tile([P, 512], f32, space="PSUM",
                                            tag="psc", name=f"psc{t}_{n}")
                                    for n in range(2)]
                            for fi, (f0, fl) in enumerate(FSH_CH):
                                for n in range(2):
                                    nc.tensor.matmul(
                                        pscs[n][:],
                                        act_sh[fi][:, t * P:(t + 1) * P],
                                        swd_sb[fi][:, n * 512:(n + 1) * 512],
                                        start=(fi == 0), stop=False)
                            for cp_ in range(3):
                                ca = slice(2 * cp_, 2 * cp_ + 2)
                                last = cp_ == 2
                                for n in range(2):
                                    if not last:
                                        nc.tensor.matmul(
                                            pscs[n][:],
                                            selct3[:, ca, t * P:(t + 1) * P],
                                            y3[:, ca, n * 512:(n + 1) * 512],
                                            start=False, stop=False,
                                            perf_mode=DR)
                                    else:
                                        nc.tensor.matmul(
                                            pscs[n][:],
                                            selct3[:, ST - 1,
                                                   t * P:(t + 1) * P],
                                            y3[:, ST - 1,
                                               n * 512:(n + 1) * 512],
                                            start=False, stop=True)
                            for n in range(2):
                                # yout = psc/32 (shared swd carries x32)
                                nc.scalar.mul(
                                    yout[:, n * 512:(n + 1) * 512],
                                    pscs[n][:], 1.0 / 32.0)
                            nc.sync.dma_start(
                                partial[t * P:(t + 1) * P, :], yout[:])

    nc.compile()
    return nc


def _get_nc():
    if "nc" not in _BUILD_CACHE:
        _BUILD_CACHE["nc"] = _build_nc()
    return _BUILD_CACHE["nc"]


def _pack_dchunk(a, width):
    """[D, W] -> [128, DC*W] with d-chunk-major free layout."""
    d, w = a.shape
    return np.ascontiguousarray(
        a.reshape(d // P, P, w).transpose(1, 0, 2).reshape(P, (d // P) * w))


def kernel(x, router_w, router_b, w_gate, w_up, w_down, sw_gate, sw_up,
           sw_down, _trace=False):
    import ml_dtypes
    f16 = np.float16
    f8 = np.dtype(ml_dtypes.float8_e4m3)
    from concourse.bass_utils import run_bass_kernel_spmd

    x = np.asarray(x, np.float32)
    x2 = np.ascontiguousarray(x.reshape(N, D))
    xp = np.ascontiguousarray(
        x2.reshape(NT, P, D).transpose(1, 0, 2)).astype(f8)
    xT_ = np.ascontiguousarray(x2.T)  # [D, N]
    # group-major packing: [P, NG, DC, GT*P]
    xtp = np.ascontiguousarray(
        xT_.reshape(DC, P, NG, GT * P).transpose(1, 2, 0, 3)
        .reshape(P, DC * N)).astype(f16)
    rwp = _pack_dchunk(np.asarray(router_w, np.float32), E).astype(f16)
    rbp = np.asarray(router_b, np.float32).reshape(1, E).astype(f16)
    wg = np.asarray(w_gate, np.float32)
    wu = np.asarray(w_up, np.float32)
    wdn = np.asarray(w_down, np.float32) * 4.0  # acts carry 16x, psy=64y
    swg = np.asarray(sw_gate, np.float32)
    swu = np.asarray(sw_up, np.float32)
    swd = np.asarray(sw_down, np.float32)

    iotaE = np.broadcast_to(np.arange(E, dtype=np.float32), (P, E)).copy()
    iotaS = np.broadcast_to(np.arange(NSLOT, dtype=f16), (P, NSLOT)).copy()
    identf = np.eye(P, dtype=np.float32)
    pcol = np.arange(P, dtype=np.float32).reshape(P, 1)

    in_maps = []
    for m in range(NCORES):
        fs = slice(m * FSH, (m + 1) * FSH)
        wh_list = []
        for e in range(EPC):
            ge = wg[m * EPC + e] * WSC
            ue = wu[m * EPC + e] * WSC
            for (a, b) in ((0, 768), (768, F)):
                fhw = b - a
                cat = np.concatenate([ge[:, a:b], ue[:, a:b]], axis=1)
                # [D, 2fhw] -> [128, DC, HW2] (per-d-chunk, padded rows)
                p3 = cat.reshape(DC, P, 2 * fhw).transpose(1, 0, 2)
                if 2 * fhw < HW2:
                    p3 = np.pad(p3, ((0, 0), (0, 0), (0, HW2 - 2 * fhw)))
                wh_list.append(p3)
        whp_ = np.ascontiguousarray(np.stack(wh_list)).astype(f8)
        wdp_ = np.ascontiguousarray(np.stack(
            [wdn[m * EPC + e].reshape(FC, P, D).transpose(1, 0, 2)
             for e in range(EPC)])).astype(f8)
        in_maps.append({
            "xp": xp,
            "xtp": xtp,
            "rwp": rwp,
            "rbp": rbp,
            "whp": whp_,
            "wdp": wdp_,
            "swgp": _pack_dchunk(swg[:, fs], FSH).astype(f16),
            "swup": _pack_dchunk(swu[:, fs], FSH).astype(f16),
            "swdp": np.ascontiguousarray(swd[fs, :] * 32.0).astype(f16),
            "coff": np.full((P, 1), float(m * EPC), np.float32),
            "pcol": pcol,
            "iotaE": iotaE,
            "iotaS": iotaS,
            "identf": identf,
        })

    nc = _get_nc()
    res = run_bass_kernel_spmd(nc, in_maps, core_ids=list(range(NCORES)),
                               trace=_trace)
    out = x2.copy()
    for r in res.results:
        out += r["partial"].astype(np.float32)
    if _trace:
        kernel._last_results = res
    return out.reshape(x.shape)


# revision 54
# speedup vs baseline: 1.0559x; 1.0163x over previous
"""DeepseekMoE Trainium2 Bass kernel (8-core expert-parallel, v5).

kernel(**inputs) takes FULL unsharded inputs (as produced by setup_inputs)
and returns the FULL output [1, 2048, 1024] fp32.

Sharding (8 cores):
  - Expert-parallel: 2 of 16 experts per core.
  - Shared expert: F-dim sliced 1408/8=176 per core (partial sums).
  - Router replicated per core.
  - Host: out = x + sum(per-core partials).

v5 design:
  - fp16 compute, fp8(e4m3) DoubleRow matmuls for dispatch and expert
    gate/up (weights pre-scaled x64 on host; the descale rides through
    silu's scale arg and a 1/64 fold into w_down).
  - Zero gpsimd usage (its tensor ops are ~20x slow AND stall the vector
    engine); iotas/identity are host inputs.
  - Sel (token-major 0/1) built per tile with scalar-engine 2-pass
    relu(1-|iota-gf0|) (even tiles) or vector is_equal (odd tiles),
    + 1 vector STT ((iota==gf1)+sel0).
  - SelCT (slot-major, prob-weighted) built directly via STT with the
    per-partition slot id as scalar, against PE-broadcast gf/w rows;
    scheduled inside the dispatch window, off the critical path.
  - Routing streamed in 2 groups of 8 token tiles (chunked in-place scan).
  - Dispatch as 4 passes (expert x d-half), 4 PSUM banks each, fp8
    DoubleRow over token-tile pairs, streaming behind Sel production.
  - Expert weights host-packed fp8: one DMA descriptor per (expert,
    F-half) for gate+up, one fp16 per expert for down; streamed through
    2-buffer pools.
  - Partial output in fp16.
"""
import numpy as np

# ---- problem constants (hardcoded; kernel.py must be self-contained) ----
N = 2048          # tokens
D = 1024          # model dim
E = 16            # experts
F = 1408          # expert ffn dim
C = 320           # per-expert capacity = ceil(1.25 * N*K / E)
NCORES = 8
EPC = E // NCORES  # experts per core = 2
FSH = F // NCORES  # shared-expert F slice = 176
P = 128
NT = N // P        # 16 token tiles
DC = D // P        # 8 d-chunks
FC = F // P        # 11 f-chunks
NSLOT = EPC * C    # 640 slots per core
TRASH = NSLOT      # sentinel slot id (matches nothing in iota 0..639)
ST = NSLOT // P    # 5 slot tiles
NG = 2             # routing groups
GT = NT // NG      # tiles per group = 8
FH = (768, 640)    # expert F halves (6 + 5 chunks of 128)
HW2 = 2 * 768      # per-d-chunk packed row width (h1 padded to match)
WSC = 64.0         # fp8 weight pre-scale (keeps w in e4m3 normal range)

_BUILD_CACHE = {}


def _build_nc():
    import concourse.bacc as bacc
    import concourse.mybir as mybir
    import concourse.tile as tile

    f32 = mybir.dt.float32
    fp16 = mybir.dt.float16
    fp8 = mybir.dt.float8e4
    u32 = mybir.dt.uint32
    Alu = mybir.AluOpType
    Act = mybir.ActivationFunctionType
    DR = mybir.MatmulPerfMode.DoubleRow

    nc = bacc.Bacc("TRN2", target_bir_lowering=False, debug=False)

    # ---- I/O (all host-packed) ----
    xp = nc.dram_tensor("xp", [P, NT, D], fp8, kind="ExternalInput").ap()
    xtp = nc.dram_tensor("xtp", [P, DC * N], fp16, kind="ExternalInput").ap()
    rwp = nc.dram_tensor("rwp", [P, DC * E], fp16, kind="ExternalInput").ap()
    rbp = nc.dram_tensor("rbp", [1, E], fp16, kind="ExternalInput").ap()
    whp = nc.dram_tensor("whp", [EPC * 2, P, DC, HW2], fp8,
                         kind="ExternalInput").ap()
    wdp = nc.dram_tensor("wdp", [EPC, P, FC, D], fp8,
                         kind="ExternalInput").ap()
    swgp = nc.dram_tensor("swgp", [P, DC * FSH], fp16,
                          kind="ExternalInput").ap()
    swup = nc.dram_tensor("swup", [P, DC * FSH], fp16,
                          kind="ExternalInput").ap()
    swdp = nc.dram_tensor("swdp", [FSH, D], fp16, kind="ExternalInput").ap()
    coff = nc.dram_tensor("coff", [P, 1], f32, kind="ExternalInput").ap()
    pcol = nc.dram_tensor("pcol", [P, 1], f32, kind="ExternalInput").ap()
    iotaE = nc.dram_tensor("iotaE", [P, E], f32, kind="ExternalInput").ap()
    iotaS = nc.dram_tensor("iotaS", [P, NSLOT], fp16,
                           kind="ExternalInput").ap()
    identf = nc.dram_tensor("identf", [P, P], f32, kind="ExternalInput").ap()
    partial = nc.dram_tensor("partial", [N, D], fp16,
                             kind="ExternalOutput").ap()

    FSH_CH = [(0, P), (P, FSH - P)]        # shared-expert f chunks: 128 + 48
    # per-expert slot sub-chunks (offset within expert, rows) and the
    # global y-tile/row they land in
    ECH = {0: [(0, 128, 0, 0), (128, 128, 1, 0), (256, 64, 2, 0)],
           1: [(0, 64, 2, 64), (64, 128, 3, 0), (192, 128, 4, 0)]}

    with tile.TileContext(nc) as tc:
        with tc.tile_pool(name="persist", bufs=1) as pp, \
             tc.tile_pool(name="stream", bufs=4) as sp:

            # ---- tiles for constants (DMAs issued after xT below) ----
            ident = pp.tile([P, P], f32, tag="ident")
            iota_e = pp.tile([P, E], f32, tag="iota_e")
            iota_h = pp.tile([P, NSLOT], fp16, tag="iota_h")
            coff_t = pp.tile([P, 1], f32, tag="coff_t")
            pcol_t = pp.tile([P, 1], f32, tag="pcol_t")
            rw_sb = pp.tile([P, DC * E], fp16, tag="rw_sb")
            rb_sb = pp.tile([1, E], fp16, tag="rb_sb")
            ones_row = pp.tile([1, P], fp16, tag="ones_row")
            nc.vector.memset(ones_row[:], 1.0)
            swg_sb = pp.tile([P, DC * FSH], fp16, tag="swg_sb")
            swu_sb = pp.tile([P, DC * FSH], fp16, tag="swu_sb")
            swd_sb = [pp.tile([fl, D], fp16, tag=f"swd_{f0}",
                              name=f"swd_{f0}") for (f0, fl) in FSH_CH]

            # routing staging [128, NT] (column = token tile), f32
            d01s = pp.tile([P, NT], f32, tag="d01s")
            idx0s = pp.tile([P, NT], f32, tag="idx0s")
            idx1s = pp.tile([P, NT], f32, tag="idx1s")
            pos0s = pp.tile([P, NT], f32, tag="pos0s")
            pos1s = pp.tile([P, NT], f32, tag="pos1s")
            gf0s = pp.tile([P, NT], f32, tag="gf0s")
            gf1s = pp.tile([P, NT], f32, tag="gf1s")
            ngf0s = pp.tile([P, NT], f32, tag="ngf0s")
            w0s = pp.tile([P, NT], f32, tag="w0s")
            w1s = pp.tile([P, NT], f32, tag="w1s")

            eq0s = [pp.tile([P, E], f32, tag=f"eq0_{t}", name=f"eq0_{t}")
                    for t in range(NT)]
            eq1s = [pp.tile([P, E], f32, tag=f"eq1_{t}", name=f"eq1_{t}")
                    for t in range(NT)]
            identh = pp.tile([P, P], fp16, tag="identh")
            # fp16 counts: exact to 2048; values above stay > capacity mask
            ohT = pp.tile([E, N], fp16, tag="ohT")
            cum = ohT  # scan runs in place (chunked, with carry)
            zcol = pp.tile([E, 1], f32, tag="zcol")
            nc.vector.memset(zcol[:], 0.0)
            hT3 = pp.tile([P, DC, NSLOT], fp8, tag="hT3")
            act_sh = [pp.tile([fl, N], fp16, tag=f"actsh_{f0}",
                              name=f"actsh_{f0}") for (f0, fl) in FSH_CH]
            selct3 = pp.tile([P, ST, N], fp8, tag="selct3")
            y3 = pp.tile([P, ST, D], fp8, tag="y3")
            repT = [pp.tile([P, N], fp16, tag=f"repT{i}",
                            name=f"repT{i}") for i in range(4)]
            icols = [pp.tile([P, 1], f32, tag=f"icol{c}",
                             name=f"icol{c}") for c in range(ST)]

            def emit_selct(c):
                s0_ = sp.tile([P, N], fp16, tag="s0", bufs=1,
                              name=f"s0_{c}")
                nc.vector.scalar_tensor_tensor(
                    out=s0_[:], in0=repT[0][:],
                    scalar=icols[c][:, 0:1], in1=repT[2][:],
                    op0=Alu.is_equal, op1=Alu.mult)
                s1_ = sp.tile([P, N], fp16, tag="s1", bufs=1,
                              name=f"s1_{c}")
                nc.vector.scalar_tensor_tensor(
                    out=s1_[:], in0=repT[1][:],
                    scalar=icols[c][:, 0:1], in1=repT[3][:],
                    op0=Alu.is_equal, op1=Alu.mult)
                nc.vector.tensor_add(selct3[:, c, :], s0_[:], s1_[:])

            # ====== gate+up weight streaming pool (reused e0 -> e1) ======
            with tc.tile_pool(name="pwh", bufs=2) as pwh:
                # ============ phase R: routing + shared + dispatch =========
                with tc.tile_pool(name="px", bufs=1) as pxp:
                    xsb = pxp.tile([P, NT, D], fp8, tag="xsb")
                    with tc.tile_pool(name="pSel", bufs=1) as psel:
                        selbf = psel.tile([P, NT, NSLOT], fp8, tag="selbf")
                        with tc.tile_pool(name="pxT", bufs=1) as pxq, \
                             tc.tile_pool(name="pR", bufs=2,
                                          space="PSUM") as pR, \
                             tc.tile_pool(name="pS", bufs=1,
                                          space="PSUM") as pS, \
                             tc.tile_pool(name="pD", bufs=1,
                                          space="PSUM") as pD:
                            xT = pxq.tile([P, DC * N], fp16, tag="xT")

                            def xts(d, a, b):
                                # xT is packed group-major on host:
                                # [P, NG, DC, GT*P] flattened
                                g, off = divmod(a, GT * P)
                                base = (g * DC + d) * GT * P
                                return xT[:, base + off:base + off + b - a]
                            # DMA priority: xT group 0 (router-critical)
                            # -> small consts -> xT group 1 -> x ->
                            # expert-0 weights
                            nc.sync.dma_start(xT[:, 0:DC * GT * P],
                                              xtp[:, 0:DC * GT * P])
                            nc.sync.dma_start(rw_sb[:], rwp)
                            nc.sync.dma_start(rb_sb[:], rbp)
                            nc.sync.dma_start(ident[:], identf)
                            nc.sync.dma_start(iota_e[:], iotaE)
                            nc.sync.dma_start(iota_h[:], iotaS)
                            nc.sync.dma_start(coff_t[:], coff)
                            nc.sync.dma_start(pcol_t[:], pcol)
                            nc.vector.tensor_copy(identh[:], ident[:])
                            nc.sync.dma_start(
                                xT[:, DC * GT * P:2 * DC * GT * P],
                                xtp[:, DC * GT * P:2 * DC * GT * P])
                            nc.sync.dma_start(xsb[:], xp)
                            nc.sync.dma_start(swg_sb[:], swgp)
                            nc.sync.dma_start(swu_sb[:], swup)
                            for (f0, fl), sd_ in zip(FSH_CH, swd_sb):
                                nc.sync.dma_start(sd_[:],
                                                  swdp[f0:f0 + fl, :])
                            wh0 = [pwh.tile([P, DC, HW2], fp8, tag="wh",
                                            name="wh00"),
                                   pwh.tile([P, DC, HW2], fp8, tag="wh",
                                            name="wh01")]
                            nc.sync.dma_start(wh0[0][:], whp[0])
                            nc.sync.dma_start(wh0[1][:], whp[1])

                            def emit_shared(fi, n):
                                f0, fl = FSH_CH[fi]
                                psg = pS.tile([P, 512], f32, space="PSUM",
                                              tag="psg")
                                psu = pS.tile([P, 512], f32, space="PSUM",
                                              tag="psu")
                                for d in range(DC):
                                    nc.tensor.matmul(
                                        psg[:fl, :],
                                        swg_sb[:, d * FSH + f0:
                                               d * FSH + f0 + fl],
                                        xts(d, n * 512, (n + 1) * 512),
                                        start=(d == 0), stop=(d == DC - 1))
                                    nc.tensor.matmul(
                                        psu[:fl, :],
                                        swu_sb[:, d * FSH + f0:
                                               d * FSH + f0 + fl],
                                        xts(d, n * 512, (n + 1) * 512),
                                        start=(d == 0), stop=(d == DC - 1))
                                sga = sp.tile([P, 512], fp16, tag="sga",
                                              bufs=2)
                                nc.scalar.activation(
                                    sga[:fl, :], psg[:fl, :], Act.Silu)
                                nc.vector.tensor_tensor(
                                    out=act_sh[fi][:, n * 512:(n + 1) * 512],
                                    in0=sga[:fl, :], in1=psu[:fl, :],
                                    op=Alu.mult)

                            # ---- routing, streamed by group; shared-expert
                            # chunks interleaved as PE filler while the
                            # vector engine paces top-2/positions ----
                            for g in range(NG):
                                t0g = g * GT
                                for t in range(t0g, t0g + GT):
                                    pt = pR.tile([P, 512], f32, space="PSUM",
                                                 tag="pt")
                                    psl = pt[:, 0:E]
                                    for d in range(DC):
                                        nc.tensor.matmul(
                                            psl,
                                            xts(d, t * P, (t + 1) * P),
                                            rw_sb[:, d * E:(d + 1) * E],
                                            start=(d == 0), stop=False)
                                    nc.tensor.matmul(
                                        psl, ones_row[:], rb_sb[:],
                                        start=False, stop=True)
                                    mx = sp.tile([P, 8], f32, tag="mx")
                                    nc.vector.max(mx[:], psl)
                                    mi = sp.tile([P, 8], u32, tag="mi")
                                    nc.vector.max_index(mi[:], mx[:], psl)
                                    nc.vector.tensor_tensor(
                                        out=d01s[:, t:t + 1], in0=mx[:, 0:1],
                                        in1=mx[:, 1:2], op=Alu.subtract)
                                    nc.vector.tensor_copy(idx0s[:, t:t + 1],
                                                          mi[:, 0:1])
                                    nc.vector.tensor_copy(idx1s[:, t:t + 1],
                                                          mi[:, 1:2])
                                    nc.vector.tensor_scalar(
                                        out=eq0s[t][:], in0=iota_e[:],
                                        scalar1=idx0s[:, t:t + 1],
                                        scalar2=None, op0=Alu.is_equal)
                                    nc.vector.tensor_scalar(
                                        out=eq1s[t][:], in0=iota_e[:],
                                        scalar1=idx1s[:, t:t + 1],
                                        scalar2=None, op0=Alu.is_equal)
                                    oh = sp.tile([P, E], f32, tag="oh")
                                    nc.vector.tensor_add(oh[:], eq0s[t][:],
                                                         eq1s[t][:])
                                    pso = pt[0:E, 128:256]
                                    nc.tensor.transpose(pso, oh[:], ident[:])
                                    nc.scalar.copy(
                                        ohT[:, t * P:(t + 1) * P], pso)

                                ini = (0.0 if g == 0
                                       else cum[:, t0g * P - 1:t0g * P])
                                nc.vector.tensor_tensor_scan(
                                    cum[:, t0g * P:(t0g + GT) * P],
                                    ohT[:, t0g * P:(t0g + GT) * P],
                                    zcol[:, 0:1].to_broadcast([E, GT * P]),
                                    ini, op0=Alu.add, op1=Alu.add)

                                for t in range(t0g, t0g + GT):
                                    pt2 = pR.tile([P, 512], f32, space="PSUM",
                                                  tag="pt")
                                    pcp = pt2[:, 0:E // 2].bitcast(fp16)
                                    nc.tensor.transpose(
                                        pcp, cum[:, t * P:(t + 1) * P],
                                        identh[0:E, 0:E])
                                    cumP = sp.tile([P, E], f32, tag="cumP")
                                    nc.scalar.copy(cumP[:], pcp)
                                    scr = sp.tile([P, E], f32, tag="scr")
                                    nc.vector.tensor_mul(scr[:], eq0s[t][:],
                                                         cumP[:])
                                    nc.vector.reduce_sum(
                                        pos0s[:, t:t + 1], scr[:],
                                        axis=mybir.AxisListType.X)
                                    scr2 = sp.tile([P, E], f32, tag="scr2")
                                    nc.vector.tensor_mul(scr2[:], eq1s[t][:],
                                                         cumP[:])
                                    nc.vector.reduce_sum(
                                        pos1s[:, t:t + 1], scr2[:],
                                        axis=mybir.AxisListType.X)

                                # ---- slot ids + weights for this group ----
                                gs = slice(t0g, t0g + GT)
                                nc.scalar.activation(w0s[:, gs], d01s[:, gs],
                                                     Act.Sigmoid)
                                nc.vector.tensor_scalar(
                                    out=w1s[:, gs], in0=w0s[:, gs],
                                    scalar1=-1.0, scalar2=1.0,
                                    op0=Alu.mult, op1=Alu.add)
                                for (idxs, poss, gfs_) in (
                                        (idx0s, pos0s, gf0s),
                                        (idx1s, pos1s, gf1s)):
                                    loc = sp.tile([P, GT], f32, tag="loc")
                                    nc.vector.tensor_scalar(
                                        out=loc[:], in0=idxs[:, gs],
                                        scalar1=coff_t[:, 0:1], scalar2=None,
                                        op0=Alu.subtract)
                                    pm1 = sp.tile([P, GT], f32, tag="pm1")
                                    nc.vector.tensor_scalar_add(
                                        pm1[:], poss[:, gs], -1.0)
                                    gr = sp.tile([P, GT], f32, tag="gr")
                                    nc.vector.scalar_tensor_tensor(
                                        out=gr[:], in0=loc[:],
                                        scalar=float(C), in1=pm1[:],
                                        op0=Alu.mult, op1=Alu.add)
                                    b1 = sp.tile([P, GT], f32, tag="b1")
                                    nc.vector.tensor_scalar(
                                        out=b1[:], in0=gr[:], scalar1=-0.5,
                                        scalar2=None, op0=Alu.is_gt)
                                    b2 = sp.tile([P, GT], f32, tag="b2")
                                    nc.vector.tensor_scalar(
                                        out=b2[:], in0=gr[:],
                                        scalar1=float(NSLOT) - 0.5,
                                        scalar2=None, op0=Alu.is_lt)
                                    b3 = sp.tile([P, GT], f32, tag="b3")
                                    nc.vector.tensor_scalar(
                                        out=b3[:], in0=pm1[:],
                                        scalar1=float(C) - 0.5,
                                        scalar2=None, op0=Alu.is_lt)
                                    val = sp.tile([P, GT], f32, tag="val")
                                    nc.vector.tensor_mul(val[:], b1[:], b2[:])
                                    nc.vector.tensor_mul(val[:], val[:],
                                                         b3[:])
                                    gm = sp.tile([P, GT], f32, tag="gm")
                                    nc.vector.tensor_scalar_add(
                                        gm[:], gr[:], -float(TRASH))
                                    nc.vector.tensor_mul(gm[:], gm[:], val[:])
                                    nc.vector.tensor_scalar_add(
                                        gfs_[:, gs], gm[:], float(TRASH))
                                nc.vector.tensor_scalar(
                                    out=ngf0s[:, gs], in0=gf0s[:, gs],
                                    scalar1=-1.0, scalar2=None, op0=Alu.mult)

                                # ---- Sel build (token-major 0/1) ----
                                # alternate k0-onehot between scalar engine
                                # (2-pass abs/relu) and vector (is_equal)
                                for t in range(t0g, t0g + GT):
                                    if t % 2 == 0:
                                        ab = sp.tile([P, NSLOT], fp16,
                                                     tag="ab", bufs=2)
                                        nc.scalar.activation(
                                            ab[:], iota_h[:], Act.Abs,
                                            bias=ngf0s[:, t:t + 1], scale=1.0)
                                        sel0 = sp.tile([P, NSLOT], fp16,
                                                       tag="sel0", bufs=2)
                                        nc.scalar.activation(
                                            sel0[:], ab[:], Act.Relu,
                                            bias=1.0, scale=-1.0)
                                    else:
                                        sel0 = sp.tile([P, NSLOT], fp16,
                                                       tag="sel0", bufs=2)
                                        nc.vector.tensor_scalar(
                                            out=sel0[:], in0=iota_h[:],
                                            scalar1=gf0s[:, t:t + 1],
                                            scalar2=None, op0=Alu.is_equal)
                                    nc.vector.scalar_tensor_tensor(
                                        out=selbf[:, t, :], in0=iota_h[:],
                                        scalar=gf1s[:, t:t + 1], in1=sel0[:],
                                        op0=Alu.is_equal, op1=Alu.add)

                            # ---- shared expert gate/up (PE filler) ----
                            for fi in range(2):
                                for n in range(4):
                                    emit_shared(fi, n)

                            # ---- dispatch: 4 passes (expert, d-half),
                            # fp8 DoubleRow over token-tile pairs ----
                            def emit_dispatch(e, dh):
                                psh = [pD.tile([P, C], f32, space="PSUM",
                                               tag=f"psh{j}",
                                               name=f"psh{e}{dh}{j}")
                                       for j in range(4)]
                                for tp in range(NT // 2):
                                    for j in range(4):
                                        d = dh * 4 + j
                                        nc.tensor.matmul(
                                            psh[j][:],
                                            xsb[:, 2 * tp:2 * tp + 2,
                                                d * P:(d + 1) * P],
                                            selbf[:, 2 * tp:2 * tp + 2,
                                                  e * C:(e + 1) * C],
                                            start=(tp == 0),
                                            stop=(tp == NT // 2 - 1),
                                            perf_mode=DR)
                                for j in range(4):
                                    d = dh * 4 + j
                                    nc.scalar.copy(
                                        hT3[:, d, e * C:(e + 1) * C],
                                        psh[j][:])

                            emit_dispatch(0, 0)

                            # ====== gf/w broadcast rows (PE work lands
                            # between dispatch passes; SelCT STTs are
                            # deferred into the expert phase) ======
                            for c in range(ST):
                                nc.vector.tensor_scalar_add(
                                    icols[c][:], pcol_t[:], float(c * P))
                            for i, src in enumerate((gf0s, gf1s, w0s, w1s)):
                                pgt = pR.tile([P, 512], f32, space="PSUM",
                                              tag="pt")
                                nc.tensor.transpose(pgt[0:NT, 0:P], src[:],
                                                    ident[:])
                                g16 = sp.tile([NT, P], fp16, tag="g16")
                                nc.scalar.copy(g16[:], pgt[0:NT, 0:P])
                                rowb = psel.tile([1, N], fp16, tag="rowb",
                                                 bufs=1, name=f"rowb{i}")
                                # scalar-engine DMA ring: stays clear of
                                # the big weight loads on the sync ring
                                nc.scalar.dma_start(rowb[:], g16[:])
                                for q in range(4):
                                    pgo = pR.tile([P, 512], f32,
                                                  space="PSUM", tag="pt")
                                    nc.tensor.matmul(
                                        pgo[:], ones_row[:],
                                        rowb[0:1, q * 512:(q + 1) * 512],
                                        start=True, stop=True)
                                    nc.vector.tensor_copy(
                                        repT[i][:, q * 512:(q + 1) * 512],
                                        pgo[:])

                            emit_dispatch(0, 1)
                            emit_dispatch(1, 0)
                            emit_dispatch(1, 1)
                        # pxT + psum pools closed (xT freed)
                    # pSel closed (selbf, repT freed)
                # px closed (xsb freed)

                # prefetch expert-1 gate/up (waits on e0 buffer release)
                wh1 = [pwh.tile([P, DC, HW2], fp8, tag="wh", name="wh10"),
                       pwh.tile([P, DC, HW2], fp8, tag="wh", name="wh11")]
                nc.sync.dma_start(wh1[0][:], whp[2])
                nc.sync.dma_start(wh1[1][:], whp[3])
                whs = [wh0, wh1]

                with tc.tile_pool(name="pwd", bufs=2) as pwd:
                    wds = [pwd.tile([P, FC, D], fp8, tag="wdt",
                                    name=f"wd{e}") for e in range(EPC)]
                    nc.sync.dma_start(wds[0][:], wdp[0])
                    nc.sync.dma_start(wds[1][:], wdp[1])

                    # ================= expert MLPs =================
                    pact_cm = tc.tile_pool(name="pact", bufs=16)
                    pact = pact_cm.__enter__()
                    with tc.tile_pool(name="pE", bufs=2,
                                      space="PSUM") as pE, \
                         tc.tile_pool(name="pY", bufs=4,
                                      space="PSUM") as pY:
                        for e in range(EPC):
                            acts3 = pact.tile([P, FC, C], fp8, tag="act3")
                            # fp8 DoubleRow over d-chunk pairs; g/u
                            # alternate two PSUM banks (same-bank
                            # back-to-back runs at half rate)
                            for fi in range(FC):
                                h = 0 if fi < 6 else 1
                                fj = fi - 6 * h
                                fhw = FH[h]
                                psg = pE.tile([P, C], f32, space="PSUM",
                                              tag="psg")
                                psu = pE.tile([P, C], f32, space="PSUM",
                                              tag="psu")
                                for dp in range(DC // 2):
                                    ds = slice(2 * dp, 2 * dp + 2)
                                    go = fj * P
                                    uo = fhw + fj * P
                                    nc.tensor.matmul(
                                        psg[:], whs[e][h][:, ds, go:go + P],
                                        hT3[:, ds, e * C:(e + 1) * C],
                                        start=(dp == 0),
                                        stop=(dp == DC // 2 - 1),
                                        perf_mode=DR)
                                    nc.tensor.matmul(
                                        psu[:], whs[e][h][:, ds, uo:uo + P],
                                        hT3[:, ds, e * C:(e + 1) * C],
                                        start=(dp == 0),
                                        stop=(dp == DC // 2 - 1),
                                        perf_mode=DR)
                                sga = sp.tile([P, C], fp16, tag="esga",
                                              bufs=2)
                                nc.scalar.activation(sga[:], psg[:],
                                                     Act.Silu,
                                                     scale=1.0 / WSC)
                                # acts_dev = 16*act: silu(g) * (64u) / 4;
                                # w_down carries a matching x4 so psy=64*y
                                nc.vector.scalar_tensor_tensor(
                                    out=acts3[:, fi, :], in0=psu[:],
                                    scalar=0.25, in1=sga[:],
                                    op0=Alu.mult, op1=Alu.mult)
                            # deferred SelCT builds (vector) slotted where
                            # they overlap PE down/g-u work
                            if e == 0:
                                emit_selct(0)
                                emit_selct(1)
                            else:
                                emit_selct(3)
                                emit_selct(4)
                            # down-projection -> y tiles (slot-major),
                            # two interleaved PSUM banks
                            groups = [(n, ch) for n in range(2)
                                      for ch in ECH[e]]
                            for gp in range(0, len(groups), 2):
                                pair = groups[gp:gp + 2]
                                psys = [pY.tile([P, 512], f32, space="PSUM",
                                                tag="psy",
                                                name=f"psy{e}_{gp}_{i}")
                                        for i in range(len(pair))]
                                for fp_ in range(6):
                                    fda = slice(2 * fp_, 2 * fp_ + 2)
                                    last = fp_ == 5
                                    for i, (n, (s0_, sl, yc, yr)) in \
                                            enumerate(pair):
                                        # DR requires dst partition base 0;
                                        # the offset-64 chunk runs regular
                                        if yr == 0 and not last:
                                            nc.tensor.matmul(
                                                psys[i][yr:yr + sl, :],
                                                acts3[:, fda, s0_:s0_ + sl],
                                                wds[e][:, fda, n * 512:
                                                       (n + 1) * 512],
                                                start=(fp_ == 0), stop=False,
                                                perf_mode=DR)
                                        elif yr == 0:
                                            nc.tensor.matmul(
                                                psys[i][yr:yr + sl, :],
                                                acts3[:, FC - 1,
                                                      s0_:s0_ + sl],
                                                wds[e][:, FC - 1, n * 512:
                                                       (n + 1) * 512],
                                                start=False, stop=True)
                                        else:
                                            for fo in (0, 1) if not last \
                                                    else (0,):
                                                fi_ = (FC - 1 if last
                                                       else 2 * fp_ + fo)
                                                nc.tensor.matmul(
                                                    psys[i][yr:yr + sl, :],
                                                    acts3[:, fi_,
                                                          s0_:s0_ + sl],
                                                    wds[e][:, fi_, n * 512:
                                                           (n + 1) * 512],
                                                    start=(fp_ == 0
                                                           and fo == 0),
                                                    stop=last)
                                for i, (n, (s0_, sl, yc, yr)) in \
                                        enumerate(pair):
                                    # y3 = 32*y = psy/2 (scalar engine)
                                    nc.scalar.mul(
                                        y3[yr:yr + sl, yc,
                                           n * 512:(n + 1) * 512],
                                        psys[i][yr:yr + sl, :], 0.5)
                            if e == 0:
                                emit_selct(2)
                    pact_cm.__exit__(None, None, None)

                    # ====== combine: out_t = shared_down + SelCT_t^T @ y ====
                    with tc.tile_pool(name="pcx", bufs=3) as pcx, \
                         tc.tile_pool(name="pC", bufs=4, space="PSUM") as pC:
                        for t in range(NT):
                            yout = pcx.tile([P, D], fp16, tag="yout")
                            pscs = [pC.tile([P, 512], f32, space="PSUM",
                                            tag="psc", name=f"psc{t}_{n}")
                                    for n in range(2)]
                            for fi, (f0, fl) in enumerate(FSH_CH):
                                for n in range(2):
                                    nc.tensor.matmul(
                                        pscs[n][:],
                                        act_sh[fi][:, t * P:(t + 1) * P],
                                        swd_sb[fi][:, n * 512:(n + 1) * 512],
                                        start=(fi == 0), stop=False)
                            for cp_ in range(3):
                                ca = slice(2 * cp_, 2 * cp_ + 2)
                                last = cp_ == 2
                                for n in range(2):
                                    if not last:
                                        nc.tensor.matmul(
                                            pscs[n][:],
                                            selct3[:, ca, t * P:(t + 1) * P],
                                            y3[:, ca, n * 512:(n + 1) * 512],
                                            start=False, stop=False,
                                            perf_mode=DR)
                                    else:
                                        nc.tensor.matmul(
                                            pscs[n][:],
                                            selct3[:, ST - 1,
                                                   t * P:(t + 1) * P],
                                            y3[:, ST - 1,
                                               n * 512:(n + 1) * 512],
                                            start=False, stop=True)
                            for n in range(2):
                                # yout = psc/32 (shared swd carries x32)
                                nc.scalar.mul(
                                    yout[:, n * 512:(n + 1) * 512],
                                    pscs[n][:], 1.0 / 32.0)
                            nc.sync.dma_start(
                                partial[t * P:(t + 1) * P, :], yout[:])

    nc.compile()
    return nc


def _get_nc():
    if "nc" not in _BUILD_CACHE:
        _BUILD_CACHE["nc"] = _build_nc()
    return _BUILD_CACHE["nc"]


def _pack_dchunk(a, width):
    """[D, W] -> [128, DC*W] with d-chunk-major free layout."""
    d, w = a.shape
    return np.ascontiguousarray(
        a.reshape(d // P, P, w).transpose(1, 0, 2).reshape(P, (d // P) * w))


def kernel(x, router_w, router_b, w_gate, w_up, w_down, sw_gate, sw_up,
           sw_down, _trace=False):
    import ml_dtypes
    f16 = np.float16
    f8 = np.dtype(ml_dtypes.float8_e4m3)
    from concourse.bass_utils import run_bass_kernel_spmd

    x = np.asarray(x, np.float32)
    x2 = np.ascontiguousarray(x.reshape(N, D))
    xp = np.ascontiguousarray(
        x2.reshape(NT, P, D).transpose(1, 0, 2)).astype(f8)
    xT_ = np.ascontiguousarray(x2.T)  # [D, N]
    # group-major packing: [P, NG, DC, GT*P]
    xtp = np.ascontiguousarray(
        xT_.reshape(DC, P, NG, GT * P).transpose(1, 2, 0, 3)
        .reshape(P, DC * N)).astype(f16)
    rwp = _pack_dchunk(np.asarray(router_w, np.float32), E).astype(f16)
    rbp = np.asarray(router_b, np.float32).reshape(1, E).astype(f16)
    wg = np.asarray(w_gate, np.float32)
    wu = np.asarray(w_up, np.float32)
    wdn = np.asarray(w_down, np.float32) * 4.0  # acts carry 16x, psy=64y
    swg = np.asarray(sw_gate, np.float32)
    swu = np.asarray(sw_up, np.float32)
    swd = np.asarray(sw_down, np.float32)

    iotaE = np.broadcast_to(np.arange(E, dtype=np.float32), (P, E)).copy()
    iotaS = np.broadcast_to(np.arange(NSLOT, dtype=f16), (P, NSLOT)).copy()
    identf = np.eye(P, dtype=np.float32)
    pcol = np.arange(P, dtype=np.float32).reshape(P, 1)

    in_maps = []
    for m in range(NCORES):
        fs = slice(m * FSH, (m + 1) * FSH)
        wh_list = []
        for e in range(EPC):
            ge = wg[m * EPC + e] * WSC
            ue = wu[m * EPC + e] * WSC
            for (a, b) in ((0, 768), (768, F)):
                fhw = b - a
                cat = np.concatenate([ge[:, a:b], ue[:, a:b]], axis=1)
                # [D, 2fhw] -> [128, DC, HW2] (per-d-chunk, padded rows)
                p3 = cat.reshape(DC, P, 2 * fhw).transpose(1, 0, 2)
                if 2 * fhw < HW2:
                    p3 = np.pad(p3, ((0, 0), (0, 0), (0, HW2 - 2 * fhw)))
                wh_list.append(p3)
        whp_ = np.ascontiguousarray(np.stack(wh_list)).astype(f8)
        wdp_ = np.ascontiguousarray(np.stack(
            [wdn[m * EPC + e].reshape(FC, P, D).transpose(1, 0, 2)
             for e in range(EPC)])).astype(f8)
        in_maps.append({
            "xp": xp,
            "xtp": xtp,
            "rwp": rwp,
            "rbp": rbp,
            "whp": whp_,
            "wdp": wdp_,
            "swgp": _pack_dchunk(swg[:, fs], FSH).astype(f16),
            "swup": _pack_dchunk(swu[:, fs], FSH).astype(f16),
            "swdp": np.ascontiguousarray(swd[fs, :] * 32.0).astype(f16),
            "coff": np.full((P, 1), float(m * EPC), np.float32),
            "pcol": pcol,
            "iotaE": iotaE,
            "iotaS": iotaS,
            "identf": identf,
        })

    nc = _get_nc()
    res = run_bass_kernel_spmd(nc, in_maps, core_ids=list(range(NCORES)),
                               trace=_trace)
    out = x2.copy()
    for r in res.results:
        out += r["partial"].astype(np.float32)
    if _trace:
        kernel._last_results = res
    return out.reshape(x.shape)
